# revision 3
# baseline (speedup 1.0000x reference)
"""3-layer GAT (GATNet) on 8 Trainium2 NeuronCores via Bass/Tile.

Sharding: nodes (and their incident edges, grouped by destination) are
partitioned across the 8 cores; weights are replicated.

v2 structure (vs v1: per-layer dense -> AllGather -> agg):
  - Layer-1 dense is computed REPLICATED on every core (the input x is
    fully known everywhere), writing the full gather table locally.
    This removes AllGather #1 (~670us on HW) at the cost of ~8x dense-1
    FLOPs (cheap).  A tiny sharded pass computes the core-local s_dst
    column (stab1) used by the per-edge side gather.
  - agg(L) and dense(L+1) are fused per 128-destination block: the
    aggregation epilogue immediately feeds the next layer's matmul and
    the table-row pack, so layer-2's shard rows stream out while agg-1
    is still running.
  - The layer-2 table AllGather is split into two half-shard
    collectives (separate shard/table tensors so the Tile dependency
    tracker allows the first to fire after block 24), overlapping the
    first half with the remaining agg-1 blocks.
  - The gather-table row split for int16 dma_gather indices is by
    per-core row halves (rows 0-3199 of each core's shard in table A,
    3200-6271 in table B), matching the split-AllGather layout.

Aggregation per 128-destination block (as v1): dma_gather the source
rows of the block's edges ([h | 1 | s_src | pad], 1280 B rows), a small
side gather for per-edge s_dst, w = exp(leaky_relu(s_src+s_dst)),
selection matrix Se_w[j,d] = (dst_local[j]==d)*w[j] built by DVE against
an iota row, and PSUM[d,:] += Se_w^T @ G on the PE; the constant-1 row
column makes the same matmuls produce the softmax denominators.
"""

import os
import sys

import numpy as np

sys.path.insert(0, "/opt/trn_rl_repo")

from contextlib import ExitStack  # noqa: E402

from concourse import bacc, bass, mybir, tile  # noqa: E402
from concourse.bass_utils import run_bass_kernel_spmd  # noqa: E402
from concourse.masks import make_identity  # noqa: E402

F32 = mybir.dt.float32
I16 = mybir.dt.int16
I32 = mybir.dt.int32

P = 128
NCORES = 8
NEG_ATT = 0.2
NEG_ACT = 0.01


def _ceil(a, b):
    return -(-a // b)


class Cfg:
    def __init__(self, n_nodes=50000, in_ch=128, hid=256, out_ch=16, graph=50,
                 **_ignored):
        assert in_ch == 128
        self.n_nodes = n_nodes
        self.in_ch = in_ch
        self.hid = hid
        self.out_ch = out_ch
        self.graph = graph
        self.nb = _ceil(_ceil(n_nodes, NCORES), P)   # dst blocks per core
        self.rows = self.nb * P                      # padded rows per core
        self.ntot = self.rows * NCORES
        self.row12 = 320          # f32: 1280 B (row bytes % 256 == 0)
        self.row3 = 64            # f32: 256 B
        # layer-1/2 table halves: per-core rows [0, RA) -> table A,
        # [RA, rows) -> table B.  Both halves' global row indices stay
        # within int16 for dma_gather.
        self.nba = 25                    # blocks per core in half A
        self.ra = self.nba * P           # 3200
        self.rb = self.rows - self.ra    # 3072
        self.na = self.ra * NCORES       # 25600 rows in table A
        self.nbt = self.rb * NCORES      # 24576 rows in table B
        assert self.na - self.ra + self.ra - 1 <= 32767
        assert self.nbt - 1 <= 32767
        # layer-3 table split (single shared table, global row split)
        self.split3 = min(32768, ((self.ntot // 2) // P) * P)
        assert self.ntot - self.split3 <= 32768


# ----------------------------------------------------------------------------
# host-side edge preprocessing
# ----------------------------------------------------------------------------

def preprocess(edge_src, edge_dst, cfg: Cfg):
    """Build the per-core edge-stream arrays.

    Layers 1/2: edge slot layout per (core, dst-block): region A
    (src%rows < ra, i.e. the source row lives in table A) occupies slots
    [0, TA*128), region B slots [TA*128, (TA+TB)*128).  Slot s maps to
    gathered-tile position (partition s%128, tile s//128).  Padding
    slots use gather index 0 and dst_local 999 (never matches).
    Layer 3: only edges into the zero-mask (output) nodes, region split
    at global row split3."""
    src = np.asarray(edge_src).astype(np.int64)
    dst = np.asarray(edge_dst).astype(np.int64)
    R, NB = cfg.rows, cfg.nb
    RA, RB = cfg.ra, cfg.rb
    N = cfg.n_nodes

    core = dst // R
    blk = (dst - core * R) // P
    s_core = src // R
    s_loc = src - s_core * R
    reg = (s_loc >= RA).astype(np.int64)
    # gather index within table A / table B
    gval = np.where(reg == 0, s_core * RA + s_loc,
                    s_core * RB + (s_loc - RA))

    # ---- layers 1/2 stream: group edges by (core, block, region) ----
    key = (core * NB + blk) * 2 + reg
    order = np.argsort(key, kind="stable")
    ks, gs, ds = key[order], gval[order], dst[order]
    starts = np.searchsorted(ks, np.arange(2 * NCORES * NB))
    pos = np.arange(len(ks)) - starts[ks]

    nA = np.zeros((NCORES, NB), np.int64)
    nB = np.zeros((NCORES, NB), np.int64)
    np.add.at(nA, (core, blk), 1 - reg)
    np.add.at(nB, (core, blk), reg)
    TA = max(1, _ceil(int(nA.max()), P))
    TB = max(1, _ceil(int(nB.max()), P))
    T = TA + TB

    c_s = ks // (2 * NB)
    b_s = (ks // 2) % NB
    r_s = ks % 2
    slot = np.where(r_s == 0, pos, TA * P + pos)
    p_s = slot % P
    t_s = slot // P

    gidx16 = np.zeros((NCORES, NB, 16, T * 8), np.int16)
    dloc16 = np.zeros((NCORES, NB, 16, T * 8), np.int16)
    dstcol = np.full((NCORES, NB, P, T), 999.0, np.float32)

    wcol = np.where(r_s == 0, pos // 16, TA * 8 + pos // 16)
    gidx16[c_s, b_s, pos % 16, wcol] = gs.astype(np.int16)

    rloc = ds - c_s * R
    dloc16[c_s, b_s, slot % 16, slot // 16] = rloc.astype(np.int16)
    dstcol[c_s, b_s, p_s, t_s] = (rloc % P).astype(np.float32)

    gidx = np.tile(gidx16, (1, 1, 8, 1))  # replicate across the 8 Q7 groups
    dloc = np.tile(dloc16, (1, 1, 8, 1))
    # pack [gidx | dloc | dstcol-as-i16] so one DMA per block loads all
    epack = np.concatenate(
        [gidx, dloc, dstcol.view(np.int16)], axis=-1)

    # ---- layer-3 stream: only edges into the zero-mask (output) nodes ----
    SPLIT3 = cfg.split3
    nodes = np.arange(N)
    zmask_node = (nodes % cfg.graph) == 0
    zcounts = np.bincount(nodes[zmask_node] // R, minlength=NCORES)
    zslot_of = np.full(N, -1, np.int64)
    for c in range(NCORES):
        zn = nodes[zmask_node & (nodes // R == c)]
        zslot_of[zn] = np.arange(len(zn))

    sel = zmask_node[dst]
    s3, d3 = src[sel], dst[sel]
    c3 = d3 // R
    r3 = (s3 >= SPLIT3).astype(np.int64)
    key3 = c3 * 2 + r3
    o3 = np.argsort(key3, kind="stable")
    k3, s3, d3 = key3[o3], s3[o3], d3[o3]
    starts3 = np.searchsorted(k3, np.arange(2 * NCORES))
    pos3 = np.arange(len(k3)) - starts3[k3]
    n3A = np.zeros(NCORES, np.int64)
    n3B = np.zeros(NCORES, np.int64)
    np.add.at(n3A, c3, 1 - r3)
    np.add.at(n3B, c3, r3)
    T3A = max(1, _ceil(int(n3A.max()), P))
    T3B = max(1, _ceil(int(n3B.max()), P))
    T3 = T3A + T3B

    cc3 = k3 // 2
    rr3 = k3 % 2
    slot3 = np.where(rr3 == 0, pos3, T3A * P + pos3)
    p3 = slot3 % P
    t3 = slot3 // P
    gidx316 = np.zeros((NCORES, 16, T3 * 8), np.int16)
    dloc316 = np.zeros((NCORES, 16, T3 * 8), np.int16)
    dstcol3 = np.full((NCORES, P, T3), 999.0, np.float32)
    wcol3 = np.where(rr3 == 0, pos3 // 16, T3A * 8 + pos3 // 16)
    val3 = np.where(rr3 == 0, s3, s3 - SPLIT3).astype(np.int16)
    gidx316[cc3, pos3 % 16, wcol3] = val3
    rl3 = d3 - cc3 * R
    dloc316[cc3, slot3 % 16, slot3 // 16] = rl3.astype(np.int16)
    dstcol3[cc3, p3, t3] = zslot_of[d3].astype(np.float32)
    gidx3 = np.tile(gidx316, (1, 8, 1))
    dloc3 = np.tile(dloc316, (1, 8, 1))
    epack3 = np.concatenate(
        [gidx3, dloc3, dstcol3.view(np.int16)], axis=-1)

    return dict(TA=TA, TB=TB, T3A=T3A, T3B=T3B,
                epack=epack, epack3=epack3,
                zcounts=zcounts)


# ----------------------------------------------------------------------------
# program builder
# ----------------------------------------------------------------------------

def build_program(cfg: Cfg, TA, TB, T3A, T3B, repeat=1):
    NB, R = cfg.nb, cfg.rows
    NBA = cfg.nba
    RA, RB = cfg.ra, cfg.rb
    NTA, NTB = cfg.na, cfg.nbt
    T, T3 = TA + TB, T3A + T3B
    ROW, ROW3 = cfg.row12, cfg.row3
    HID, OUT = cfg.hid, cfg.out_ch
    NTOT = cfg.ntot
    SPLIT3 = cfg.split3
    NBTOT = NB * NCORES

    nc = bacc.Bacc("TRN2", target_bir_lowering=False, debug=False,
                   num_devices=NCORES)

    # ---- I/O ----
    xTf = nc.dram_tensor("xTf", [P, NTOT], F32, kind="ExternalInput")
    xTl = nc.dram_tensor("xTl", [P, R], F32, kind="ExternalInput")
    W1e = nc.dram_tensor("W1e", [P, HID + 3], F32, kind="ExternalInput")
    W2e = nc.dram_tensor("W2e", [HID, HID + 3], F32, kind="ExternalInput")
    W3e = nc.dram_tensor("W3e", [HID, OUT + 3], F32, kind="ExternalInput")
    b1 = nc.dram_tensor("b1", [1, HID], F32, kind="ExternalInput")
    b2 = nc.dram_tensor("b2", [1, HID], F32, kind="ExternalInput")
    b3 = nc.dram_tensor("b3", [1, OUT], F32, kind="ExternalInput")
    epack = nc.dram_tensor("epack", [NB, P, T * 18], I16,
                           kind="ExternalInput")
    epack3 = nc.dram_tensor("epack3", [P, T3 * 18], I16,
                            kind="ExternalInput")
    out_d = nc.dram_tensor("out", [P, OUT], F32, kind="ExternalOutput")

    # ---- internal DRAM ----
    # layer-1 table: local full copy (replicated dense)
    h1tabA = nc.dram_tensor("h1tabA", [NTA, ROW], F32)
    h1tabB = nc.dram_tensor("h1tabB", [NTB, ROW], F32)
    # layer-2 shard halves + gathered tables
    h2shardA = nc.dram_tensor("h2shardA", [RA, ROW], F32)
    h2shardB = nc.dram_tensor("h2shardB", [RB, ROW], F32)
    h2tabA = nc.dram_tensor("h2tabA", [NTA, ROW], F32, addr_space="Shared")
    h2tabB = nc.dram_tensor("h2tabB", [NTB, ROW], F32, addr_space="Shared")
    h3shard = nc.dram_tensor("h3shard", [R, ROW3], F32)
    h3tab = nc.dram_tensor("h3tab", [NTOT, ROW3], F32, addr_space="Shared")
    SROW = 64
    stab = [nc.dram_tensor(f"stab{i}", [R, SROW], F32) for i in (1, 2, 3)]

    rg = [list(range(NCORES))]

    with tile.TileContext(nc) as tc, ExitStack() as ctx:
        cpool = ctx.enter_context(tc.tile_pool(name="const", bufs=1))
        wpool = ctx.enter_context(tc.tile_pool(name="weights", bufs=1))
        xf_pool = ctx.enter_context(tc.tile_pool(name="xchunk", bufs=2))
        lt_pool = ctx.enter_context(tc.tile_pool(name="lhsT", bufs=4))
        row_pool = ctx.enter_context(tc.tile_pool(name="rows", bufs=3))
        idx_pool = ctx.enter_context(tc.tile_pool(name="idx", bufs=4))
        g_pool = ctx.enter_context(tc.tile_pool(name="gather", bufs=2))
        s_pool = ctx.enter_context(tc.tile_pool(name="scal", bufs=4))
        se_pool = ctx.enter_context(tc.tile_pool(name="sew", bufs=3))
        a_pool = ctx.enter_context(tc.tile_pool(name="arow", bufs=3))
        ps_dense = ctx.enter_context(
            tc.tile_pool(name="psd", bufs=2, space="PSUM"))
        ps_agg = ctx.enter_context(
            tc.tile_pool(name="psa", bufs=2, space="PSUM"))
        ps_tp = ctx.enter_context(
            tc.tile_pool(name="pst", bufs=2, space="PSUM"))
        ps_bc = ctx.enter_context(
            tc.tile_pool(name="psb", bufs=1, space="PSUM"))

        # constants
        ident = cpool.tile([P, P], F32, tag="ident")
        make_identity(nc, ident[:])
        TMAX = max(T, T3)
        iota_i = cpool.tile([P, TMAX * P], I32, tag="iotai")
        nc.gpsimd.iota(iota_i[:], pattern=[[0, TMAX], [1, P]], base=0,
                       channel_multiplier=0)
        iota_f = cpool.tile([P, TMAX * P], F32, tag="iotaf")
        nc.vector.tensor_copy(out=iota_f[:], in_=iota_i[:])
        ones1 = cpool.tile([1, P], F32, tag="ones1")
        nc.vector.memset(ones1[:], 1.0)

        # preload weights
        w1_sb = wpool.tile([P, HID + 3], F32, tag="w1")
        nc.sync.dma_start(out=w1_sb[:], in_=W1e[:, :])
        w2_sb = [wpool.tile([P, HID + 3], F32, tag=f"w2_{k}",
                            name=f"w2sb{k}") for k in range(2)]
        for k in range(2):
            nc.sync.dma_start(out=w2_sb[k][:], in_=W2e[k * P:(k + 1) * P, :])
        w3_sb = [wpool.tile([P, OUT + 3], F32, tag=f"w3_{k}",
                            name=f"w3sb{k}") for k in range(2)]
        for k in range(2):
            nc.sync.dma_start(out=w3_sb[k][:], in_=W3e[k * P:(k + 1) * P, :])

        def bias_bcast(bd, C, tag):
            brow = cpool.tile([1, C], F32, tag=f"brow_{tag}")
            nc.sync.dma_start(out=brow[:], in_=bd[:, :])
            bps = ps_bc.tile([P, C], F32, tag="bps")
            nc.tensor.matmul(bps[:], lhsT=ones1[:], rhs=brow[:],
                             start=True, stop=True)
            bbc = cpool.tile([P, C], F32, tag=f"bbc_{tag}")
            nc.vector.tensor_copy(out=bbc[:], in_=bps[:])
            return bbc

        def pack_row(row_ap, ps, ncols):
            """row = [h | 1 | s_src | 0-pad]; ps cols [0:ncols]=h,
            ncols=0 (becomes 1.0), ncols+1=s_src, ncols+2=s_dst."""
            nc.vector.tensor_copy(out=row_ap[:, 0:ncols + 2],
                                  in_=ps[:, 0:ncols + 2])
            nc.vector.memset(row_ap[:, ncols:ncols + 1], 1.0)

        def pack_srow(srow_ap, ps, sd_col):
            nc.vector.memset(srow_ap[:, 1:SROW], 0.0)
            nc.vector.tensor_copy(out=srow_ap[:, 0:1],
                                  in_=ps[:, sd_col:sd_col + 1])

        # ------------------------------------------------------------------
        def dense1_mini():
            """Sharded pass: stab1 (core-local s_dst1) only."""
            GRP = 8
            for g0 in range(0, NB, GRP):
                gn = min(GRP, NB - g0)
                srow = row_pool.tile([P, GRP * SROW], F32, tag="d1row")
                for i in range(gn):
                    it = g0 + i
                    lt = lt_pool.tile([P, P], F32, tag="mxT")
                    nc.sync.dma_start(out=lt[:],
                                      in_=xTl[:, it * P:(it + 1) * P])
                    ps = ps_dense.tile([P, 1], F32, tag="dps")
                    nc.tensor.matmul(ps[:], lhsT=lt[:],
                                     rhs=w1_sb[:, HID + 2:HID + 3],
                                     start=True, stop=True)
                    nc.vector.memset(srow[:, i * SROW + 1:(i + 1) * SROW],
                                     0.0)
                    nc.vector.tensor_copy(
                        out=srow[:, i * SROW:i * SROW + 1], in_=ps[:])
                dst = stab[0][g0 * P:(g0 + gn) * P, :]
                nc.sync.dma_start(
                    out=dst.rearrange("(c p) f -> p c f", p=P),
                    in_=srow[:, 0:gn * SROW].rearrange(
                        "p (c f) -> p c f", f=SROW))

        # ------------------------------------------------------------------
        def dense1_full():
            """Replicated dense-1: every core computes the whole table."""
            XC = 25  # blocks per xTf chunk load
            for c0 in range(0, NBTOT, XC):
                cn = min(XC, NBTOT - c0)
                xch = xf_pool.tile([P, XC * P], F32, tag="xch")
                nc.sync.dma_start(out=xch[:, 0:cn * P],
                                  in_=xTf[:, c0 * P:(c0 + cn) * P])
                # row-write groups within this chunk (chunk = half A of
                # one core when cn==25; the tail groups handle half B)
                g = c0
                while g < c0 + cn:
                    core_i = g // NB
                    bl = g % NB
                    if bl < NBA:
                        gn = min(NBA - bl, c0 + cn - g, 5)
                    else:
                        gn = min(NB - bl, c0 + cn - g, 8)
                    rowt = row_pool.tile([P, 8 * ROW], F32, tag="d1row")
                    for i in range(gn):
                        it = g + i
                        ps = ps_dense.tile([P, HID + 3], F32, tag="dps")
                        nc.tensor.matmul(
                            ps[:], lhsT=xch[:, (it - c0) * P:(it - c0 + 1) * P],
                            rhs=w1_sb[:], start=True, stop=True)
                        rr = rowt[:, i * ROW:(i + 1) * ROW]
                        pack_row(rr, ps, HID)
                        nc.vector.memset(rr[:, HID + 2:ROW], 0.0)
                    bl2 = g % NB
                    if bl2 < NBA:
                        dst = h1tabA[(core_i * NBA + bl2) * P:
                                     (core_i * NBA + bl2 + gn) * P, :]
                    else:
                        b_off = core_i * (NB - NBA) + (bl2 - NBA)
                        dst = h1tabB[b_off * P:(b_off + gn) * P, :]
                    nc.sync.dma_start(
                        out=dst.rearrange("(c p) f -> p c f", p=P),
                        in_=rowt[:, 0:gn * ROW].rearrange(
                            "p (c f) -> p c f", f=ROW))
                    g += gn

        # ------------------------------------------------------------------
        def agg_dense(layer, tabs, sd_t, nblocks, tA, tB, epack_t, C_out,
                      bbc, w_next, next_cols, shards, stab_next):
            """Fused: aggregate layer L per dst block, then immediately run
            the layer L+1 dense for that block and write its table rows.

            tabs = (tabA_ap, tabB_ap) gather sources for this layer.
            shards = list of (row_limit, tensor) for next-layer shard
            halves (None for layer 3 output)."""
            tT = tA + tB
            n_mm = C_out + 1  # h columns + the constant-1 (denominator) col
            for b in range(nblocks):
                ep = idx_pool.tile([P, tT * 18], I16, tag="ep")
                if nblocks == 1:
                    nc.sync.dma_start(out=ep[:], in_=epack_t[:, :])
                else:
                    nc.sync.dma_start(out=ep[:], in_=epack_t[b, :, :])
                gi = ep[:, 0:tT * 8]
                dl = ep[:, tT * 8:tT * 16]
                dc = ep[:, tT * 16:tT * 18].bitcast(F32)
                rowlen = ROW if layer < 3 else ROW3
                G = g_pool.tile([P, tT * rowlen], F32, tag="G")
                G3d = G[:].rearrange("p (t c) -> p t c", c=rowlen)
                nc.gpsimd.dma_gather(
                    out_ap=G3d[:, 0:tA, :], in_ap=tabs[0],
                    idxs_ap=gi[:, 0:tA * 8], num_idxs=tA * P,
                    num_idxs_reg=tA * P, elem_size=rowlen, elem_step=rowlen,
                    single_packet=False)
                nc.gpsimd.dma_gather(
                    out_ap=G3d[:, tA:tT, :], in_ap=tabs[1],
                    idxs_ap=gi[:, tA * 8:tT * 8], num_idxs=tB * P,
                    num_idxs_reg=tB * P, elem_size=rowlen, elem_step=rowlen,
                    single_packet=False)
                Gs = g_pool.tile([P, tT * SROW], F32, tag="Gs")
                Gs3d = Gs[:].rearrange("p (t c) -> p t c", c=SROW)
                nc.gpsimd.dma_gather(
                    out_ap=Gs3d[:, :, :], in_ap=sd_t[:, :],
                    idxs_ap=dl, num_idxs=tT * P,
                    num_idxs_reg=tT * P, elem_size=SROW, elem_step=SROW,
                    single_packet=False)
                sdp = Gs[:, 0::SROW]
                ssrc = G[:, C_out + 1::rowlen]
                z = s_pool.tile([P, tT], F32, tag="z")
                nc.vector.tensor_tensor(out=z[:], in0=ssrc, in1=sdp,
                                        op=mybir.AluOpType.add)
                e = s_pool.tile([P, tT], F32, tag="e")
                nc.vector.scalar_tensor_tensor(
                    out=e[:], in0=z[:], scalar=NEG_ATT, in1=z[:],
                    op0=mybir.AluOpType.mult, op1=mybir.AluOpType.max)
                w = s_pool.tile([P, tT], F32, tag="w")
                nc.scalar.activation(w[:], e[:],
                                     mybir.ActivationFunctionType.Exp)
                ps = ps_agg.tile([P, n_mm], F32, tag="aps")
                swa = se_pool.tile([P, tT * P], F32, tag="swa")
                dc3 = dc.unsqueeze(-1).to_broadcast([P, tT, P])
                w3 = w[:].unsqueeze(-1).to_broadcast([P, tT, P])
                swa3 = swa[:].rearrange("p (t d) -> p t d", d=P)
                nc.vector.tensor_tensor(
                    out=swa3, in0=iota_f[:, 0:tT * P].rearrange(
                        "p (t d) -> p t d", d=P),
                    in1=dc3, op=mybir.AluOpType.is_equal)
                nc.vector.tensor_tensor(
                    out=swa3, in0=swa3, in1=w3, op=mybir.AluOpType.mult)
                for t in range(tT):
                    nc.tensor.matmul(
                        ps[:], lhsT=swa[:, t * P:(t + 1) * P],
                        rhs=G[:, t * rowlen:t * rowlen + n_mm],
                        start=(t == 0), stop=(t == tT - 1))
                dn = s_pool.tile([P, 1], F32, tag="dn")
                nc.vector.tensor_scalar_add(dn[:], ps[:, C_out:C_out + 1],
                                            1e-30)
                rc = s_pool.tile([P, 1], F32, tag="rc")
                nc.vector.reciprocal(rc[:], dn[:])
                ar = a_pool.tile([P, C_out], F32, tag="ar")
                nc.scalar.activation(ar[:], ps[:, 0:C_out],
                                     mybir.ActivationFunctionType.Copy,
                                     scale=rc[:])
                nc.vector.tensor_tensor(out=ar[:], in0=ar[:], in1=bbc[:],
                                        op=mybir.AluOpType.add)
                if layer == 3:
                    nc.sync.dma_start(out=out_d[:, :], in_=ar[:])
                    continue
                # ---- fused next-layer dense for this block ----
                ar2 = a_pool.tile([P, C_out], F32, tag="ar2")
                nc.vector.scalar_tensor_tensor(
                    out=ar2[:], in0=ar[:], scalar=NEG_ACT, in1=ar[:],
                    op0=mybir.AluOpType.mult, op1=mybir.AluOpType.max)
                psd = ps_dense.tile([P, next_cols + 3], F32, tag="dps")
                for k in range(2):
                    tp = ps_tp.tile([P, P], F32, tag="tp")
                    nc.tensor.transpose(tp[:], ar2[:, k * P:(k + 1) * P],
                                        ident[:])
                    lt = lt_pool.tile([P, P], F32, tag="flt")
                    nc.scalar.copy(out=lt[:], in_=tp[:])
                    nc.tensor.matmul(psd[:], lhsT=lt[:], rhs=w_next[k][:],
                                     start=(k == 0), stop=(k == 1))
                nrow = ROW if layer == 1 else ROW3
                row = row_pool.tile([P, nrow], F32, tag="frow")
                pack_row(row[:], psd, next_cols)
                nc.vector.memset(row[:, next_cols + 2:nrow], 0.0)
                srow = row_pool.tile([P, SROW], F32, tag="fsrow")
                pack_srow(srow[:], psd, next_cols + 2)
                if layer == 1:
                    if b < NBA:
                        nc.sync.dma_start(
                            out=shards[0][b * P:(b + 1) * P, :], in_=row[:])
                    else:
                        nc.sync.dma_start(
                            out=shards[1][(b - NBA) * P:(b - NBA + 1) * P, :],
                            in_=row[:])
                else:
                    nc.sync.dma_start(
                        out=shards[0][b * P:(b + 1) * P, :], in_=row[:])
                nc.sync.dma_start(
                    out=stab_next[b * P:(b + 1) * P, :], in_=srow[:])
                # fire the first half-AllGather as soon as half A is done
                if layer == 1 and b == NBA - 1:
                    nc.gpsimd.collective_compute(
                        "AllGather", mybir.AluOpType.bypass,
                        replica_groups=rg,
                        ins=[h2shardA.ap()], outs=[h2tabA.ap()])

        # ====================== the network ======================
        for _rep in range(repeat):
            bbc1 = bias_bcast(b1, HID, "b1")
            bbc2 = bias_bcast(b2, HID, "b2")
            bbc3 = bias_bcast(b3, OUT, "b3")
            dense1_mini()
            dense1_full()
            agg_dense(1, (h1tabA.ap(), h1tabB.ap()), stab[0], NB, TA, TB,
                      epack, HID, bbc1, w2_sb, HID,
                      (h2shardA, h2shardB), stab[1])
            nc.gpsimd.collective_compute(
                "AllGather", mybir.AluOpType.bypass, replica_groups=rg,
                ins=[h2shardB.ap()], outs=[h2tabB.ap()])
            agg_dense(2, (h2tabA.ap(), h2tabB.ap()), stab[1], NB, TA, TB,
                      epack, HID, bbc2, w3_sb, OUT,
                      (h3shard,), stab[2])
            nc.gpsimd.collective_compute(
                "AllGather", mybir.AluOpType.bypass, replica_groups=rg,
                ins=[h3shard.ap()], outs=[h3tab.ap()])
            agg_dense(3, (h3tab[0:SPLIT3, :], h3tab[SPLIT3:NTOT, :]),
                      stab[2], 1, T3A, T3B, epack3, OUT, bbc3,
                      None, 0, None, None)

    nc.compile()
    return nc


# ----------------------------------------------------------------------------
# host wrapper
# ----------------------------------------------------------------------------

def make_in_maps(inputs, pre, cfg: Cfg):
    R = cfg.rows
    N = cfg.n_nodes
    NTOT = cfg.ntot
    x = np.asarray(inputs["x"], np.float32)

    def wext(W, a_s, a_d):
        W = np.asarray(W, np.float32)
        a_s = np.asarray(a_s, np.float32)
        a_d = np.asarray(a_d, np.float32)
        z = np.zeros((W.shape[0], 1), np.float32)
        return np.concatenate(
            [W, z, (W @ a_s)[:, None], (W @ a_d)[:, None]], axis=1
        ).astype(np.float32)

    W1e = wext(inputs["W1"], inputs["a_src1"], inputs["a_dst1"])
    W2e = wext(inputs["W2"], inputs["a_src2"], inputs["a_dst2"])
    W3e = wext(inputs["W3"], inputs["a_src3"], inputs["a_dst3"])
    b1 = np.asarray(inputs["b1"], np.float32).reshape(1, -1)
    b2 = np.asarray(inputs["b2"], np.float32).reshape(1, -1)
    b3 = np.asarray(inputs["b3"], np.float32).reshape(1, -1)
    xf = np.zeros((P, NTOT), np.float32)
    xf[:, 0:N] = x.T
    in_maps = []
    for c in range(NCORES):
        lo, hi = c * R, min((c + 1) * R, N)
        xs = np.zeros((P, R), np.float32)
        xs[:, 0:max(0, hi - lo)] = x[lo:hi].T
        in_maps.append({
            "xTf": xf, "xTl": xs, "W1e": W1e, "W2e": W2e, "W3e": W3e,
            "b1": b1, "b2": b2, "b3": b3,
            "epack": pre["epack"][c], "epack3": pre["epack3"][c],
        })
    return in_maps


_CACHE = {}


def get_program(cfg: Cfg, TA, TB, T3A, T3B, repeat=1):
    key = (cfg.n_nodes, TA, TB, T3A, T3B, repeat)
    if key not in _CACHE:
        _CACHE[key] = build_program(cfg, TA, TB, T3A, T3B, repeat)
    return _CACHE[key]


def run(inputs, cfg: Cfg, trace=False):
    pre = preprocess(inputs["edge_src"], inputs["edge_dst"], cfg)
    in_maps = make_in_maps(inputs, pre, cfg)
    nc = get_program(cfg, pre["TA"], pre["TB"], pre["T3A"], pre["T3B"])
    res = run_bass_kernel_spmd(nc, in_maps, list(range(NCORES)), trace=trace)
    outs = []
    for c in range(NCORES):
        outs.append(res.results[c]["out"][0:pre["zcounts"][c], :])
    return np.concatenate(outs, axis=0).astype(np.float32), res


def kernel(**inputs):
    cfg = Cfg(n_nodes=inputs["x"].shape[0],
              in_ch=inputs["x"].shape[1],
              hid=inputs["W1"].shape[1],
              out_ch=inputs["W3"].shape[1])
    out, _ = run(inputs, cfg)
    return out


# revision 8
# speedup vs baseline: 1.9553x; 1.9553x over previous
"""3-layer GAT (GATNet) on 8 Trainium2 NeuronCores via Bass/Tile.

Sharding: nodes (and their incident edges, grouped by destination) are
partitioned across the 8 cores; weights are replicated.

v3 structure:
  - Layer-1 dense is computed REPLICATED on every core (the input x is
    fully known everywhere), writing the full gather table locally --
    no AllGather for layer 1.  A tiny sharded pass computes the
    core-local per-block s_dst rows (stabT1).
  - agg(L) and dense(L+1) are fused per 128-destination block.
  - The layer-2 table AllGather is split into two half-shard
    collectives: AG2a fires mid-way through the fused agg-1 loop
    (overlapping its tail), and AG2b overlaps with agg-2's region-A
    pass, which only needs table A.  agg-2 runs as two passes
    (region-A matmuls accumulate into an SBUF stash, region-B pass
    adds on top), so no agg-2 work waits on AG2b except pass B.
  - Per-edge s_dst for layers 1/2 is computed on-chip (band matmul
    broadcast of the block's 128 s_dst values + masked reduce) instead
    of a 256 B/edge side gather.  Layer 3 keeps the side gather.
  - Layer-3 table is compacted to just the rows referenced by edges
    into output nodes (~14k of 50k) before its AllGather.

Aggregation per 128-destination block: dma_gather the source rows of
the block's edges ([h | 1 | s_src | pad], 1280 B rows),
w = exp(leaky_relu(s_src+s_dst)), selection matrix
Se_w[j,d] = (dst_local[j]==d)*w[j] built by DVE against an iota row,
and PSUM[d,:] += Se_w^T @ G on the PE; the constant-1 row column makes
the same matmuls produce the softmax denominators.
"""

import sys

import numpy as np

sys.path.insert(0, "/opt/trn_rl_repo")

from contextlib import ExitStack  # noqa: E402

from concourse import bacc, bass, mybir, tile  # noqa: E402
from concourse.bass_utils import run_bass_kernel_spmd  # noqa: E402
from concourse.masks import make_identity  # noqa: E402

F32 = mybir.dt.float32
I16 = mybir.dt.int16
I32 = mybir.dt.int32

P = 128
NCORES = 8
NEG_ATT = 0.2
NEG_ACT = 0.01
SROW = 64


def _ceil(a, b):
    return -(-a // b)


class Cfg:
    def __init__(self, n_nodes=50000, in_ch=128, hid=256, out_ch=16, graph=50,
                 **_ignored):
        assert in_ch == 128
        self.n_nodes = n_nodes
        self.in_ch = in_ch
        self.hid = hid
        self.out_ch = out_ch
        self.graph = graph
        self.nb = _ceil(_ceil(n_nodes, NCORES), P)   # dst blocks per core
        self.rows = self.nb * P                      # padded rows per core
        self.ntot = self.rows * NCORES
        self.row12 = 320          # f32: 1280 B (row bytes % 256 == 0)
        self.row3 = 64            # f32: 256 B
        # layer-1/2 table halves: per-core rows [0, RA) -> table A,
        # [RA, rows) -> table B.  Both halves' global row indices stay
        # within int16 for dma_gather.
        self.nba = 25                    # blocks per core in half A
        self.ra = self.nba * P           # 3200
        self.rb = self.rows - self.ra    # 3072
        self.na = self.ra * NCORES       # 25600 rows in table A
        self.nbt = self.rb * NCORES      # 24576 rows in table B
        assert self.na - 1 <= 32767 and self.nbt - 1 <= 32767


# ----------------------------------------------------------------------------
# host-side edge preprocessing
# ----------------------------------------------------------------------------

def _wrap16(vals16, T8):
    """[n, 16, T8] <- scatter list layout used by dma_gather idx streams."""
    return vals16


def preprocess(edge_src, edge_dst, cfg: Cfg):
    """Build the per-core edge-stream arrays.

    Layers 1/2 epack layout per block (i16 columns):
      [ gidxA (tA*8) | dcA (tA*2, i32 pairs) | gidxB (tB*8) | dcB (tB*2) ]
    so agg-2's pass A DMAs cols [0, tA*10) and pass B the rest.
    Slot s of a region maps to gathered-tile position (partition s%128,
    tile s//128).  Padding slots use gather index 0 and dst 999.
    Layer 3: compact table; epack3 keeps the v1 layout
    [gidx3 | dloc3 | dstcol3] with regions split at half the compact
    table."""
    src = np.asarray(edge_src).astype(np.int64)
    dst = np.asarray(edge_dst).astype(np.int64)
    R, NB = cfg.rows, cfg.nb
    RA, RB = cfg.ra, cfg.rb
    N = cfg.n_nodes

    core = dst // R
    blk = (dst - core * R) // P
    s_core = src // R
    s_loc = src - s_core * R
    reg = (s_loc >= RA).astype(np.int64)
    gval = np.where(reg == 0, s_core * RA + s_loc,
                    s_core * RB + (s_loc - RA))

    key = (core * NB + blk) * 2 + reg
    order = np.argsort(key, kind="stable")
    ks, gs, ds = key[order], gval[order], dst[order]
    starts = np.searchsorted(ks, np.arange(2 * NCORES * NB))
    pos = np.arange(len(ks)) - starts[ks]

    nA = np.zeros((NCORES, NB), np.int64)
    nB = np.zeros((NCORES, NB), np.int64)
    np.add.at(nA, (core, blk), 1 - reg)
    np.add.at(nB, (core, blk), reg)
    TA = max(1, _ceil(int(nA.max()), P))
    TB = max(1, _ceil(int(nB.max()), P))

    c_s = ks // (2 * NB)
    b_s = (ks // 2) % NB
    r_s = ks % 2

    gidxA = np.zeros((NCORES, NB, 16, TA * 8), np.int16)
    gidxB = np.zeros((NCORES, NB, 16, TB * 8), np.int16)
    dcA = np.full((NCORES, NB, P, TA), 999, np.int32)
    dcB = np.full((NCORES, NB, P, TB), 999, np.int32)

    mA = r_s == 0
    gidxA[c_s[mA], b_s[mA], pos[mA] % 16, pos[mA] // 16] = \
        gs[mA].astype(np.int16)
    gidxB[c_s[~mA], b_s[~mA], pos[~mA] % 16, pos[~mA] // 16] = \
        gs[~mA].astype(np.int16)
    rloc = ds - c_s * R
    dcA[c_s[mA], b_s[mA], pos[mA] % P, pos[mA] // P] = \
        (rloc[mA] % P).astype(np.int32)
    dcB[c_s[~mA], b_s[~mA], pos[~mA] % P, pos[~mA] // P] = \
        (rloc[~mA] % P).astype(np.int32)

    epack = np.concatenate(
        [np.tile(gidxA, (1, 1, 8, 1)), dcA.view(np.int16).reshape(
            NCORES, NB, P, TA * 2),
         np.tile(gidxB, (1, 1, 8, 1)), dcB.view(np.int16).reshape(
            NCORES, NB, P, TB * 2)], axis=-1)

    # ---- layer-3: compact table of rows referenced by z-edges ----
    nodes = np.arange(N)
    zmask_node = (nodes % cfg.graph) == 0
    zcounts = np.bincount(nodes[zmask_node] // R, minlength=NCORES)
    zslot_of = np.full(N, -1, np.int64)
    for c in range(NCORES):
        zn = nodes[zmask_node & (nodes // R == c)]
        zslot_of[zn] = np.arange(len(zn))

    sel = zmask_node[dst]
    s3, d3 = src[sel], dst[sel]
    # needed local rows per source core (dedup), compact slot assignment
    s3core = s3 // R
    s3loc = s3 - s3core * R
    need_lists = []
    np3 = 0
    for c in range(NCORES):
        u = np.unique(s3loc[s3core == c])
        need_lists.append(u)
        np3 = max(np3, len(u))
    NP3 = _ceil(np3, P) * P
    cmpidx16 = np.zeros((NCORES, 16, NP3 // 16), np.int16)
    cslot_of = np.zeros((NCORES, R), np.int64)
    for c in range(NCORES):
        u = need_lists[c]
        cmpidx16[c, np.arange(len(u)) % 16, np.arange(len(u)) // 16] = \
            u.astype(np.int16)
        cslot_of[c, u] = np.arange(len(u))
    cmpidx = np.tile(cmpidx16, (1, 8, 1))
    # compact global row of each z-edge source
    g3 = s3core * NP3 + cslot_of[s3core, s3loc]
    SPLIT3 = 4 * NP3
    assert NCORES * NP3 - SPLIT3 - 1 <= 32767 and SPLIT3 - 1 <= 32767

    c3 = d3 // R
    r3 = (g3 >= SPLIT3).astype(np.int64)
    key3 = c3 * 2 + r3
    o3 = np.argsort(key3, kind="stable")
    k3, g3o, d3o = key3[o3], g3[o3], d3[o3]
    starts3 = np.searchsorted(k3, np.arange(2 * NCORES))
    pos3 = np.arange(len(k3)) - starts3[k3]
    n3A = np.zeros(NCORES, np.int64)
    n3B = np.zeros(NCORES, np.int64)
    np.add.at(n3A, c3, 1 - r3)
    np.add.at(n3B, c3, r3)
    T3A = max(1, _ceil(int(n3A.max()), P))
    T3B = max(1, _ceil(int(n3B.max()), P))
    T3 = T3A + T3B

    cc3 = k3 // 2
    rr3 = k3 % 2
    slot3 = np.where(rr3 == 0, pos3, T3A * P + pos3)
    gidx316 = np.zeros((NCORES, 16, T3 * 8), np.int16)
    dloc316 = np.zeros((NCORES, 16, T3 * 8), np.int16)
    dstcol3 = np.full((NCORES, P, T3), 999.0, np.float32)
    wcol3 = np.where(rr3 == 0, pos3 // 16, T3A * 8 + pos3 // 16)
    val3 = np.where(rr3 == 0, g3o, g3o - SPLIT3).astype(np.int16)
    gidx316[cc3, pos3 % 16, wcol3] = val3
    rl3 = d3o - cc3 * R
    dloc316[cc3, slot3 % 16, slot3 // 16] = rl3.astype(np.int16)
    dstcol3[cc3, slot3 % P, slot3 // P] = zslot_of[d3o].astype(np.float32)
    epack3 = np.concatenate(
        [np.tile(gidx316, (1, 8, 1)), np.tile(dloc316, (1, 8, 1)),
         dstcol3.view(np.int16)], axis=-1)

    return dict(TA=TA, TB=TB, T3A=T3A, T3B=T3B, NP3=NP3,
                epack=epack, epack3=epack3, cmpidx=cmpidx,
                zcounts=zcounts)


# ----------------------------------------------------------------------------
# program builder
# ----------------------------------------------------------------------------

def build_program(cfg: Cfg, TA, TB, T3A, T3B, NP3, repeat=1):
    NB, R = cfg.nb, cfg.rows
    NBA = cfg.nba
    RA, RB = cfg.ra, cfg.rb
    NTA, NTB = cfg.na, cfg.nbt
    T, T3 = TA + TB, T3A + T3B
    ROW, ROW3 = cfg.row12, cfg.row3
    HID, OUT = cfg.hid, cfg.out_ch
    SPLIT3 = 4 * NP3
    NBTOT = NB * NCORES

    nc = bacc.Bacc("TRN2", target_bir_lowering=False, debug=False,
                   num_devices=NCORES)

    # ---- I/O ----
    xTf = nc.dram_tensor("xTf", [P, NBTOT * P], F32, kind="ExternalInput")
    xTl = nc.dram_tensor("xTl", [P, R], F32, kind="ExternalInput")
    W1e = nc.dram_tensor("W1e", [P, HID + 3], F32, kind="ExternalInput")
    W2e = nc.dram_tensor("W2e", [HID, HID + 3], F32, kind="ExternalInput")
    W3e = nc.dram_tensor("W3e", [HID, OUT + 3], F32, kind="ExternalInput")
    b1 = nc.dram_tensor("b1", [1, HID], F32, kind="ExternalInput")
    b2 = nc.dram_tensor("b2", [1, HID], F32, kind="ExternalInput")
    b3 = nc.dram_tensor("b3", [1, OUT], F32, kind="ExternalInput")
    epack = nc.dram_tensor("epack", [NB, P, T * 10], I16,
                           kind="ExternalInput")
    epack3 = nc.dram_tensor("epack3", [P, T3 * 18], I16,
                            kind="ExternalInput")
    cmpidx = nc.dram_tensor("cmpidx", [P, NP3 // 16], I16,
                            kind="ExternalInput")
    out_d = nc.dram_tensor("out", [P, OUT], F32, kind="ExternalOutput")

    # ---- internal DRAM ----
    h1tabA = nc.dram_tensor("h1tabA", [NTA, ROW], F32)
    h1tabB = nc.dram_tensor("h1tabB", [NTB, ROW], F32)
    h2shardA = nc.dram_tensor("h2shardA", [RA, ROW], F32)
    h2shardB = nc.dram_tensor("h2shardB", [RB, ROW], F32)
    h2tabA = nc.dram_tensor("h2tabA", [NTA, ROW], F32, addr_space="Shared")
    h2tabB = nc.dram_tensor("h2tabB", [NTB, ROW], F32, addr_space="Shared")
    h3shard = nc.dram_tensor("h3shard", [R, ROW3], F32)
    h3cshard = nc.dram_tensor("h3cshard", [NP3, ROW3], F32)
    h3ctab = nc.dram_tensor("h3ctab", [NCORES * NP3, ROW3], F32,
                            addr_space="Shared")
    stabT1 = nc.dram_tensor("stabT1", [1, NB * P], F32)
    stabT2 = nc.dram_tensor("stabT2", [1, NB * P], F32)
    acc_d = nc.dram_tensor("acc_d", [R, HID + 1], F32)
    stab3 = nc.dram_tensor("stab3", [R, SROW], F32)

    rg = [list(range(NCORES))]

    with tile.TileContext(nc) as tc, ExitStack() as ctx:
        cpool = ctx.enter_context(tc.tile_pool(name="const", bufs=1))
        wpool = ctx.enter_context(tc.tile_pool(name="weights", bufs=1))
        xf_pool = ctx.enter_context(tc.tile_pool(name="xchunk", bufs=2))
        lt_pool = ctx.enter_context(tc.tile_pool(name="lhsT", bufs=4))
        d1_pool = ctx.enter_context(tc.tile_pool(name="d1row", bufs=2))
        row_pool = ctx.enter_context(tc.tile_pool(name="rows", bufs=2))
        idx_pool = ctx.enter_context(tc.tile_pool(name="idx", bufs=4))
        g_pool = ctx.enter_context(tc.tile_pool(name="gather", bufs=3))
        s_pool = ctx.enter_context(tc.tile_pool(name="scal", bufs=4))
        se_pool = ctx.enter_context(tc.tile_pool(name="sew", bufs=3))
        tmp_pool = ctx.enter_context(tc.tile_pool(name="tmp", bufs=3))
        a_pool = ctx.enter_context(tc.tile_pool(name="arow", bufs=3))
        bnd_pool = ctx.enter_context(tc.tile_pool(name="band", bufs=2))
        sdc_pool = ctx.enter_context(tc.tile_pool(name="sdc", bufs=2))
        aio_pool = ctx.enter_context(tc.tile_pool(name="aio", bufs=2))
        ps_dense = ctx.enter_context(
            tc.tile_pool(name="psd", bufs=2, space="PSUM"))
        ps_agg = ctx.enter_context(
            tc.tile_pool(name="psa", bufs=3, space="PSUM"))
        ps_tp = ctx.enter_context(
            tc.tile_pool(name="pst", bufs=2, space="PSUM"))
        ps_bc = ctx.enter_context(
            tc.tile_pool(name="psb", bufs=1, space="PSUM"))

        # constants
        ident = cpool.tile([P, P], F32, tag="ident")
        make_identity(nc, ident[:])
        TMAX = max(T, T3)
        iota_i = cpool.tile([P, TMAX * P], I32, tag="iotai")
        nc.gpsimd.iota(iota_i[:], pattern=[[0, TMAX], [1, P]], base=0,
                       channel_multiplier=0)
        ones1 = cpool.tile([1, P], F32, tag="ones1")
        nc.vector.memset(ones1[:], 1.0)
        # f32 iota for layer-3 (dstcol3 is f32 there)
        iota_f = cpool.tile([P, T3 * P], F32, tag="iotaf")
        nc.vector.tensor_copy(out=iota_f[:], in_=iota_i[:, 0:T3 * P])

        # preload weights
        w1_sb = wpool.tile([P, HID + 3], F32, tag="w1")
        nc.sync.dma_start(out=w1_sb[:], in_=W1e[:, :])
        w2_sb = [wpool.tile([P, HID + 3], F32, tag=f"w2_{k}",
                            name=f"w2sb{k}") for k in range(2)]
        for k in range(2):
            nc.sync.dma_start(out=w2_sb[k][:], in_=W2e[k * P:(k + 1) * P, :])
        w3_sb = [wpool.tile([P, OUT + 3], F32, tag=f"w3_{k}",
                            name=f"w3sb{k}") for k in range(2)]
        for k in range(2):
            nc.sync.dma_start(out=w3_sb[k][:], in_=W3e[k * P:(k + 1) * P, :])

        def bias_bcast(bd, C, tag):
            brow = cpool.tile([1, C], F32, tag=f"brow_{tag}")
            nc.sync.dma_start(out=brow[:], in_=bd[:, :])
            bps = ps_bc.tile([P, C], F32, tag="bps")
            nc.tensor.matmul(bps[:], lhsT=ones1[:], rhs=brow[:],
                             start=True, stop=True)
            bbc = cpool.tile([P, C], F32, tag=f"bbc_{tag}")
            nc.vector.tensor_copy(out=bbc[:], in_=bps[:])
            return bbc

        def pack_row(row_ap, ps, ncols, rowlen):
            """row = [h | 1 | s_src | 0-pad]."""
            nc.vector.tensor_copy(out=row_ap[:, 0:ncols + 2],
                                  in_=ps[:, 0:ncols + 2])
            nc.vector.memset(row_ap[:, ncols:ncols + 1], 1.0)
            nc.vector.memset(row_ap[:, ncols + 2:rowlen], 0.0)

        # s_dst-row staging: collect [P,1] columns for SG consecutive
        # blocks, then transpose and DMA SG rows of stabT at once.
        SG = 8

        def sd_stage(sdc, i, ps_col):
            nc.vector.tensor_copy(out=sdc[:, i:i + 1], in_=ps_col)

        def sd_flush(sdc, stabT, g0, gn):
            tp = ps_tp.tile([P, P], F32, tag="tp")
            nc.tensor.transpose(tp[0:SG, :], sdc[:, 0:SG], ident[:])
            sr = sdc_pool.tile([SG, P], F32, tag="srows")
            nc.scalar.copy(out=sr[0:gn, :], in_=tp[0:gn, :])
            nc.sync.dma_start(
                out=stabT[0:1, g0 * P:(g0 + gn) * P].rearrange(
                    "o (a b) -> (o a) b", b=P),
                in_=sr[0:gn, :])

        # ------------------------------------------------------------------
        def dense1_mini():
            """Sharded pass: stabT1 (core-local per-block s_dst rows)."""
            for g0 in range(0, NB, SG):
                gn = min(SG, NB - g0)
                sdc = sdc_pool.tile([P, SG], F32, tag="sdc")
                for i in range(gn):
                    it = g0 + i
                    lt = lt_pool.tile([P, P], F32, tag="mxT")
                    nc.sync.dma_start(out=lt[:],
                                      in_=xTl[:, it * P:(it + 1) * P])
                    ps = ps_dense.tile([P, 1], F32, tag="dps")
                    nc.tensor.matmul(ps[:], lhsT=lt[:],
                                     rhs=w1_sb[:, HID + 2:HID + 3],
                                     start=True, stop=True)
                    sd_stage(sdc, i, ps[:, 0:1])
                sd_flush(sdc, stabT1, g0, gn)

        # ------------------------------------------------------------------
        def dense1_full():
            """Replicated dense-1: every core computes the whole table."""
            XC = 10  # blocks per xTf chunk load
            DG = 5   # blocks per table-row write group
            for c0 in range(0, NBTOT, XC):
                cn = min(XC, NBTOT - c0)
                xch = xf_pool.tile([P, XC * P], F32, tag="xch")
                nc.sync.dma_start(out=xch[:, 0:cn * P],
                                  in_=xTf[:, c0 * P:(c0 + cn) * P])
                g = c0
                while g < c0 + cn:
                    core_i = g // NB
                    bl = g % NB
                    if bl < NBA:
                        gn = min(NBA - bl, c0 + cn - g, DG)
                    else:
                        gn = min(NB - bl, c0 + cn - g, DG)
                    rowt = d1_pool.tile([P, DG * ROW], F32, tag="d1row")
                    for i in range(gn):
                        it = g + i
                        ps = ps_dense.tile([P, HID + 3], F32, tag="dps")
                        nc.tensor.matmul(
                            ps[:],
                            lhsT=xch[:, (it - c0) * P:(it - c0 + 1) * P],
                            rhs=w1_sb[:], start=True, stop=True)
                        pack_row(rowt[:, i * ROW:(i + 1) * ROW], ps, HID, ROW)
                    if bl < NBA:
                        dst = h1tabA[(core_i * NBA + bl) * P:
                                     (core_i * NBA + bl + gn) * P, :]
                    else:
                        b_off = core_i * (NB - NBA) + (bl - NBA)
                        dst = h1tabB[b_off * P:(b_off + gn) * P, :]
                    nc.sync.dma_start(
                        out=dst.rearrange("(c p) f -> p c f", p=P),
                        in_=rowt[:, 0:gn * ROW].rearrange(
                            "p (c f) -> p c f", f=ROW))
                    g += gn

        # ------------------------------------------------------------------
        BG = 4  # blocks per band-broadcast matmul

        def make_bandg(stabT, g0):
            gw = min(BG, NB - g0)
            srow_sb = bnd_pool.tile([1, BG * P], F32, tag="srow")
            nc.sync.dma_start(out=srow_sb[:, 0:gw * P],
                              in_=stabT[0:1, g0 * P:(g0 + gw) * P])
            bps = ps_bc.tile([P, BG * P], F32, tag="bps")
            nc.tensor.matmul(bps[:, 0:gw * P], lhsT=ones1[:],
                             rhs=srow_sb[:, 0:gw * P], start=True, stop=True)
            bandg = bnd_pool.tile([P, BG * P], F32, tag="bandg")
            nc.vector.tensor_copy(out=bandg[:, 0:gw * P],
                                  in_=bps[:, 0:gw * P])
            return bandg

        def region_pass(G, swa, nt, t0, gi, dcs, band, tab, rowlen, n_mm,
                        ps, start, stop):
            """Gather region tiles, build Se_w, run the matmuls.

            G/swa are tile slices sized for nt tiles; t0 is the iota
            tile offset (region B starts at tA for the combined agg-1
            pass so slot ids match dc values built per-region)."""
            G3d = G.rearrange("p (t c) -> p t c", c=rowlen)
            nc.gpsimd.dma_gather(
                out_ap=G3d, in_ap=tab, idxs_ap=gi, num_idxs=nt * P,
                num_idxs_reg=nt * P, elem_size=rowlen, elem_step=rowlen,
                single_packet=False)
            swa3 = swa.rearrange("p (t d) -> p t d", d=P)
            dc3 = dcs.unsqueeze(-1).to_broadcast([P, nt, P])
            nc.vector.tensor_tensor(
                out=swa3, in0=iota_i[:, 0:nt * P].rearrange(
                    "p (t d) -> p t d", d=P),
                in1=dc3, op=mybir.AluOpType.is_equal)
            tmp = tmp_pool.tile([P, T * P], F32, tag="tmp")
            tmp3 = tmp[:, 0:nt * P].rearrange("p (t d) -> p t d", d=P)
            band3 = band[:].unsqueeze(1).to_broadcast([P, nt, P])
            nc.vector.tensor_tensor(out=tmp3, in0=swa3, in1=band3,
                                    op=mybir.AluOpType.mult)
            sd = s_pool.tile([P, T], F32, tag="sd")
            nc.vector.tensor_reduce(out=sd[:, 0:nt], in_=tmp3,
                                    axis=mybir.AxisListType.X,
                                    op=mybir.AluOpType.add)
            ssrc = G[:, n_mm::rowlen]
            z = s_pool.tile([P, T], F32, tag="z")
            nc.vector.tensor_tensor(out=z[:, 0:nt], in0=ssrc,
                                    in1=sd[:, 0:nt],
                                    op=mybir.AluOpType.add)
            e = s_pool.tile([P, T], F32, tag="e")
            nc.vector.scalar_tensor_tensor(
                out=e[:, 0:nt], in0=z[:, 0:nt], scalar=NEG_ATT,
                in1=z[:, 0:nt],
                op0=mybir.AluOpType.mult, op1=mybir.AluOpType.max)
            w = s_pool.tile([P, T], F32, tag="w")
            nc.scalar.activation(w[:, 0:nt], e[:, 0:nt],
                                 mybir.ActivationFunctionType.Exp)
            w3 = w[:, 0:nt].unsqueeze(-1).to_broadcast([P, nt, P])
            nc.vector.tensor_tensor(out=swa3, in0=swa3, in1=w3,
                                    op=mybir.AluOpType.mult)
            for t in range(nt):
                nc.tensor.matmul(
                    ps[:], lhsT=swa[:, t * P:(t + 1) * P],
                    rhs=G[:, t * rowlen:t * rowlen + n_mm],
                    start=(start and t == 0), stop=(stop and t == nt - 1))

        # ------------------------------------------------------------------
        def epilogue(layer, src_ap, bbc, C_out, w_next, next_cols, b,
                     shards, stabT_next, sdc, stab3_t):
            """Softmax divide + bias (+ fused next dense + row pack)."""
            dn = s_pool.tile([P, 1], F32, tag="dn")
            nc.vector.tensor_scalar_add(dn[:], src_ap[:, C_out:C_out + 1],
                                        1e-30)
            rc = s_pool.tile([P, 1], F32, tag="rc")
            nc.vector.reciprocal(rc[:], dn[:])
            ar = a_pool.tile([P, C_out], F32, tag="ar")
            nc.scalar.activation(ar[:], src_ap[:, 0:C_out],
                                 mybir.ActivationFunctionType.Copy,
                                 scale=rc[:])
            nc.vector.tensor_tensor(out=ar[:], in0=ar[:], in1=bbc[:],
                                    op=mybir.AluOpType.add)
            if layer == 3:
                nc.sync.dma_start(out=out_d[:, :], in_=ar[:])
                return
            ar2 = a_pool.tile([P, C_out], F32, tag="ar2")
            nc.vector.scalar_tensor_tensor(
                out=ar2[:], in0=ar[:], scalar=NEG_ACT, in1=ar[:],
                op0=mybir.AluOpType.mult, op1=mybir.AluOpType.max)
            tps = []
            for k in range(2):
                tp = ps_tp.tile([P, P], F32, tag="tp")
                nc.tensor.transpose(tp[:], ar2[:, k * P:(k + 1) * P],
                                    ident[:])
                tps.append(tp)
            lts = []
            for k in range(2):
                lt = lt_pool.tile([P, P], F32, tag="flt")
                nc.scalar.copy(out=lt[:], in_=tps[k][:])
                lts.append(lt)
            psd = ps_dense.tile([P, next_cols + 3], F32, tag="dps")
            for k in range(2):
                nc.tensor.matmul(psd[:], lhsT=lts[k][:], rhs=w_next[k][:],
                                 start=(k == 0), stop=(k == 1))
            nrow = ROW if layer == 1 else ROW3
            row = row_pool.tile([P, nrow], F32, tag="frow")
            pack_row(row[:], psd, next_cols, nrow)
            if layer == 1:
                if b < NBA:
                    nc.sync.dma_start(
                        out=shards[0][b * P:(b + 1) * P, :], in_=row[:])
                else:
                    nc.sync.dma_start(
                        out=shards[1][(b - NBA) * P:(b - NBA + 1) * P, :],
                        in_=row[:])
                sd_stage(sdc, b % SG, psd[:, next_cols + 2:next_cols + 3])
                if b % SG == SG - 1 or b == NB - 1:
                    sd_flush(sdc, stabT_next, (b // SG) * SG,
                             b % SG + 1)
            else:
                nc.sync.dma_start(
                    out=shards[0][b * P:(b + 1) * P, :], in_=row[:])
                srow = row_pool.tile([P, SROW], F32, tag="fsrow")
                nc.vector.memset(srow[:, 1:SROW], 0.0)
                nc.vector.tensor_copy(
                    out=srow[:, 0:1],
                    in_=psd[:, next_cols + 2:next_cols + 3])
                nc.sync.dma_start(
                    out=stab3_t[b * P:(b + 1) * P, :], in_=srow[:])

        # ====================== the network ======================
        for _rep in range(repeat):
            bbc1 = bias_bcast(b1, HID, "b1")
            bbc2 = bias_bcast(b2, HID, "b2")
            bbc3 = bias_bcast(b3, OUT, "b3")
            dense1_mini()
            dense1_full()

            # ---- fused agg-1 + dense-2 ----
            sdc2 = None
            bandg = None
            for b in range(NB):
                if b % SG == 0:
                    sdc2 = sdc_pool.tile([P, SG], F32, tag="sdc")
                if b % BG == 0:
                    bandg = make_bandg(stabT1, b)
                band = bandg[:, (b % BG) * P:(b % BG + 1) * P]
                ep = idx_pool.tile([P, T * 10], I16, tag="ep")
                nc.sync.dma_start(out=ep[:], in_=epack[b, :, :])
                G = g_pool.tile([P, T * ROW], F32, tag="G")
                swa = se_pool.tile([P, T * P], F32, tag="swa")
                ps = ps_agg.tile([P, HID + 1], F32, tag="aps")
                region_pass(G[:, 0:TA * ROW], swa[:, 0:TA * P], TA, 0,
                            ep[:, 0:TA * 8],
                            ep[:, TA * 8:TA * 10].bitcast(I32),
                            band, h1tabA.ap(), ROW, HID + 1, ps,
                            True, False)
                region_pass(G[:, TA * ROW:T * ROW], swa[:, TA * P:T * P],
                            TB, TA, ep[:, TA * 10:TA * 10 + TB * 8],
                            ep[:, TA * 10 + TB * 8:T * 10].bitcast(I32),
                            band, h1tabB.ap(), ROW, HID + 1, ps,
                            False, True)
                epilogue(1, ps[:], bbc1, HID, w2_sb, HID, b,
                         (h2shardA, h2shardB), stabT2, sdc2, None)
                if b == NBA - 1:
                    nc.gpsimd.collective_compute(
                        "AllGather", mybir.AluOpType.bypass,
                        replica_groups=rg,
                        ins=[h2shardA.ap()], outs=[h2tabA.ap()])
            nc.gpsimd.collective_compute(
                "AllGather", mybir.AluOpType.bypass, replica_groups=rg,
                ins=[h2shardB.ap()], outs=[h2tabB.ap()])

            # ---- agg-2 pass A (table A only; overlaps AG2b) ----
            bandg = None
            for b in range(NB):
                if b % BG == 0:
                    bandg = make_bandg(stabT2, b)
                band = bandg[:, (b % BG) * P:(b % BG + 1) * P]
                ep = idx_pool.tile([P, TA * 10], I16, tag="ep")
                nc.sync.dma_start(out=ep[:], in_=epack[b, :, 0:TA * 10])
                G = g_pool.tile([P, T * ROW], F32, tag="G")
                swa = se_pool.tile([P, T * P], F32, tag="swa")
                ps = ps_agg.tile([P, HID + 1], F32, tag="aps")
                region_pass(G[:, 0:TA * ROW], swa[:, 0:TA * P], TA, 0,
                            ep[:, 0:TA * 8],
                            ep[:, TA * 8:TA * 10].bitcast(I32),
                            band, h2tabA.ap(), ROW, HID + 1, ps,
                            True, True)
                acw = aio_pool.tile([P, HID + 1], F32, tag="acw")
                nc.vector.tensor_copy(out=acw[:], in_=ps[:])
                nc.sync.dma_start(out=acc_d[b * P:(b + 1) * P, :],
                                  in_=acw[:])

            # ---- agg-2 pass B + fused dense-3 ----
            bandg = None
            for b in range(NB):
                if b % BG == 0:
                    bandg = make_bandg(stabT2, b)
                band = bandg[:, (b % BG) * P:(b % BG + 1) * P]
                accb = aio_pool.tile([P, HID + 1], F32, tag="acb")
                nc.sync.dma_start(out=accb[:],
                                  in_=acc_d[b * P:(b + 1) * P, :])
                ep = idx_pool.tile([P, TB * 10], I16, tag="ep")
                nc.sync.dma_start(out=ep[:],
                                  in_=epack[b, :, TA * 10:T * 10])
                G = g_pool.tile([P, T * ROW], F32, tag="G")
                swa = se_pool.tile([P, T * P], F32, tag="swa")
                ps = ps_agg.tile([P, HID + 1], F32, tag="aps")
                region_pass(G[:, 0:TB * ROW], swa[:, 0:TB * P], TB, 0,
                            ep[:, 0:TB * 8],
                            ep[:, TB * 8:TB * 10].bitcast(I32),
                            band, h2tabB.ap(), ROW, HID + 1, ps,
                            True, True)
                arp = a_pool.tile([P, HID + 1], F32, tag="arp")
                nc.vector.tensor_tensor(
                    out=arp[:], in0=ps[:], in1=accb[:],
                    op=mybir.AluOpType.add)
                epilogue(2, arp[:], bbc2, HID, w3_sb, OUT, b,
                         (h3shard,), None, None, stab3)

            # ---- compact + AllGather layer-3 table ----
            ci = idx_pool.tile([P, NP3 // 16], I16, tag="ci")
            nc.sync.dma_start(out=ci[:], in_=cmpidx[:, :])
            cmp_sb = g_pool.tile([P, (NP3 // P) * ROW3], F32, tag="G")
            cmp3d = cmp_sb[:].rearrange("p (t c) -> p t c", c=ROW3)
            nc.gpsimd.dma_gather(
                out_ap=cmp3d, in_ap=h3shard.ap(), idxs_ap=ci[:],
                num_idxs=NP3, num_idxs_reg=NP3, elem_size=ROW3,
                elem_step=ROW3, single_packet=False)
            nc.sync.dma_start(
                out=h3cshard.ap().rearrange("(t p) f -> p t f", p=P),
                in_=cmp3d)
            nc.gpsimd.collective_compute(
                "AllGather", mybir.AluOpType.bypass, replica_groups=rg,
                ins=[h3cshard.ap()], outs=[h3ctab.ap()])

            # ---- agg-3 (single block, side-gather for s_dst) ----
            ep = idx_pool.tile([P, T3 * 18], I16, tag="ep3")
            nc.sync.dma_start(out=ep[:], in_=epack3[:, :])
            gi = ep[:, 0:T3 * 8]
            dl = ep[:, T3 * 8:T3 * 16]
            dc = ep[:, T3 * 16:T3 * 18].bitcast(F32)
            G = g_pool.tile([P, T3 * ROW3], F32, tag="G")
            G3d = G[:].rearrange("p (t c) -> p t c", c=ROW3)
            nc.gpsimd.dma_gather(
                out_ap=G3d[:, 0:T3A, :], in_ap=h3ctab[0:SPLIT3, :],
                idxs_ap=gi[:, 0:T3A * 8], num_idxs=T3A * P,
                num_idxs_reg=T3A * P, elem_size=ROW3, elem_step=ROW3,
                single_packet=False)
            nc.gpsimd.dma_gather(
                out_ap=G3d[:, T3A:T3, :],
                in_ap=h3ctab[SPLIT3:NCORES * NP3, :],
                idxs_ap=gi[:, T3A * 8:T3 * 8], num_idxs=T3B * P,
                num_idxs_reg=T3B * P, elem_size=ROW3, elem_step=ROW3,
                single_packet=False)
            Gs = tmp_pool.tile([P, T3 * SROW], F32, tag="tmp")
            Gs3d = Gs[:, 0:T3 * SROW].rearrange("p (t c) -> p t c", c=SROW)
            nc.gpsimd.dma_gather(
                out_ap=Gs3d, in_ap=stab3[:, :], idxs_ap=dl,
                num_idxs=T3 * P, num_idxs_reg=T3 * P, elem_size=SROW,
                elem_step=SROW, single_packet=False)
            sdp = Gs[:, 0:T3 * SROW:SROW]
            ssrc = G[:, OUT + 1::ROW3]
            z = s_pool.tile([P, T3], F32, tag="z")
            nc.vector.tensor_tensor(out=z[:], in0=ssrc, in1=sdp,
                                    op=mybir.AluOpType.add)
            e = s_pool.tile([P, T3], F32, tag="e")
            nc.vector.scalar_tensor_tensor(
                out=e[:], in0=z[:], scalar=NEG_ATT, in1=z[:],
                op0=mybir.AluOpType.mult, op1=mybir.AluOpType.max)
            w = s_pool.tile([P, T3], F32, tag="w")
            nc.scalar.activation(w[:], e[:],
                                 mybir.ActivationFunctionType.Exp)
            ps = ps_agg.tile([P, OUT + 1], F32, tag="aps")
            swa = se_pool.tile([P, T3 * P], F32, tag="swa")
            dc3 = dc.unsqueeze(-1).to_broadcast([P, T3, P])
            w3b = w[:].unsqueeze(-1).to_broadcast([P, T3, P])
            swa3 = swa[:].rearrange("p (t d) -> p t d", d=P)
            nc.vector.tensor_tensor(
                out=swa3, in0=iota_f[:].rearrange("p (t d) -> p t d", d=P),
                in1=dc3, op=mybir.AluOpType.is_equal)
            nc.vector.tensor_tensor(out=swa3, in0=swa3, in1=w3b,
                                    op=mybir.AluOpType.mult)
            for t in range(T3):
                nc.tensor.matmul(
                    ps[:], lhsT=swa[:, t * P:(t + 1) * P],
                    rhs=G[:, t * ROW3:t * ROW3 + OUT + 1],
                    start=(t == 0), stop=(t == T3 - 1))
            epilogue(3, ps[:], bbc3, OUT, None, 0, 0, None, None, None,
                     None)

    nc.compile()
    return nc


# ----------------------------------------------------------------------------
# host wrapper
# ----------------------------------------------------------------------------

def make_in_maps(inputs, pre, cfg: Cfg):
    R = cfg.rows
    N = cfg.n_nodes
    NTOT = cfg.ntot
    x = np.asarray(inputs["x"], np.float32)

    def wext(W, a_s, a_d):
        W = np.asarray(W, np.float32)
        a_s = np.asarray(a_s, np.float32)
        a_d = np.asarray(a_d, np.float32)
        z = np.zeros((W.shape[0], 1), np.float32)
        return np.concatenate(
            [W, z, (W @ a_s)[:, None], (W @ a_d)[:, None]], axis=1
        ).astype(np.float32)

    W1e = wext(inputs["W1"], inputs["a_src1"], inputs["a_dst1"])
    W2e = wext(inputs["W2"], inputs["a_src2"], inputs["a_dst2"])
    W3e = wext(inputs["W3"], inputs["a_src3"], inputs["a_dst3"])
    b1 = np.asarray(inputs["b1"], np.float32).reshape(1, -1)
    b2 = np.asarray(inputs["b2"], np.float32).reshape(1, -1)
    b3 = np.asarray(inputs["b3"], np.float32).reshape(1, -1)
    xf = np.zeros((P, NTOT), np.float32)
    xf[:, 0:N] = x.T
    in_maps = []
    for c in range(NCORES):
        lo, hi = c * R, min((c + 1) * R, N)
        xs = np.zeros((P, R), np.float32)
        xs[:, 0:max(0, hi - lo)] = x[lo:hi].T
        in_maps.append({
            "xTf": xf, "xTl": xs, "W1e": W1e, "W2e": W2e, "W3e": W3e,
            "b1": b1, "b2": b2, "b3": b3,
            "epack": pre["epack"][c], "epack3": pre["epack3"][c],
            "cmpidx": pre["cmpidx"][c],
        })
    return in_maps


_CACHE = {}


def get_program(cfg: Cfg, TA, TB, T3A, T3B, NP3, repeat=1):
    key = (cfg.n_nodes, TA, TB, T3A, T3B, NP3, repeat)
    if key not in _CACHE:
        _CACHE[key] = build_program(cfg, TA, TB, T3A, T3B, NP3, repeat)
    return _CACHE[key]


def run(inputs, cfg: Cfg, trace=False):
    pre = preprocess(inputs["edge_src"], inputs["edge_dst"], cfg)
    in_maps = make_in_maps(inputs, pre, cfg)
    nc = get_program(cfg, pre["TA"], pre["TB"], pre["T3A"], pre["T3B"],
                     pre["NP3"])
    res = run_bass_kernel_spmd(nc, in_maps, list(range(NCORES)), trace=trace)
    outs = []
    for c in range(NCORES):
        outs.append(res.results[c]["out"][0:pre["zcounts"][c], :])
    return np.concatenate(outs, axis=0).astype(np.float32), res


def kernel(**inputs):
    cfg = Cfg(n_nodes=inputs["x"].shape[0],
              in_ch=inputs["x"].shape[1],
              hid=inputs["W1"].shape[1],
              out_ch=inputs["W3"].shape[1])
    out, _ = run(inputs, cfg)
    return out


# revision 15
# speedup vs baseline: 2.0117x; 1.0289x over previous
"""3-layer GAT (GATNet) on 8 Trainium2 NeuronCores via Bass/Tile.

Sharding: nodes (and their incident edges, grouped by destination) are
partitioned across the 8 cores; weights are replicated.

v3 structure:
  - Layer-1 dense is computed REPLICATED on every core (the input x is
    fully known everywhere), writing the full gather table locally --
    no AllGather for layer 1.  A tiny sharded pass computes the
    core-local per-block s_dst rows (stabT1).
  - agg(L) and dense(L+1) are fused per 128-destination block.
  - The layer-2 table AllGather is split into two half-shard
    collectives: AG2a fires mid-way through the fused agg-1 loop
    (overlapping its tail), and AG2b overlaps with agg-2's region-A
    pass, which only needs table A.  agg-2 runs as two passes
    (region-A matmuls accumulate into an SBUF stash, region-B pass
    adds on top), so no agg-2 work waits on AG2b except pass B.
  - Per-edge s_dst for layers 1/2 is computed on-chip (band matmul
    broadcast of the block's 128 s_dst values + masked reduce) instead
    of a 256 B/edge side gather.  Layer 3 keeps the side gather.
  - Layer-3 table is compacted to just the rows referenced by edges
    into output nodes (~14k of 50k) before its AllGather.

Aggregation per 128-destination block: dma_gather the source rows of
the block's edges ([h | 1 | s_src | pad], 1280 B rows),
w = exp(leaky_relu(s_src+s_dst)), selection matrix
Se_w[j,d] = (dst_local[j]==d)*w[j] built by DVE against an iota row,
and PSUM[d,:] += Se_w^T @ G on the PE; the constant-1 row column makes
the same matmuls produce the softmax denominators.
"""

import sys

import numpy as np

sys.path.insert(0, "/opt/trn_rl_repo")

from contextlib import ExitStack  # noqa: E402

from concourse import bacc, bass, mybir, tile  # noqa: E402
from concourse.bass_utils import run_bass_kernel_spmd  # noqa: E402
from concourse.masks import make_identity  # noqa: E402

F32 = mybir.dt.float32
I16 = mybir.dt.int16
I32 = mybir.dt.int32

P = 128
NCORES = 8
NEG_ATT = 0.2
NEG_ACT = 0.01
SROW = 64


def _ceil(a, b):
    return -(-a // b)


class Cfg:
    def __init__(self, n_nodes=50000, in_ch=128, hid=256, out_ch=16, graph=50,
                 **_ignored):
        assert in_ch == 128
        self.n_nodes = n_nodes
        self.in_ch = in_ch
        self.hid = hid
        self.out_ch = out_ch
        self.graph = graph
        self.nb = _ceil(_ceil(n_nodes, NCORES), P)   # dst blocks per core
        self.rows = self.nb * P                      # padded rows per core
        self.ntot = self.rows * NCORES
        self.row12 = 320          # f32: 1280 B (row bytes % 256 == 0)
        self.row3 = 64            # f32: 256 B
        # layer-1/2 table halves: per-core rows [0, RA) -> table A,
        # [RA, rows) -> table B.  Both halves' global row indices stay
        # within int16 for dma_gather.
        self.nba = 25                    # blocks per core in half A
        self.ra = self.nba * P           # 3200
        self.rb = self.rows - self.ra    # 3072
        self.na = self.ra * NCORES       # 25600 rows in table A
        self.nbt = self.rb * NCORES      # 24576 rows in table B
        assert self.na - 1 <= 32767 and self.nbt - 1 <= 32767


# ----------------------------------------------------------------------------
# host-side edge preprocessing
# ----------------------------------------------------------------------------

def _wrap16(vals16, T8):
    """[n, 16, T8] <- scatter list layout used by dma_gather idx streams."""
    return vals16


def preprocess(edge_src, edge_dst, cfg: Cfg):
    """Build the per-core edge-stream arrays.

    Layers 1/2 epack layout per block (i16 columns):
      [ gidxA (tA*8) | dcA (tA*2, i32 pairs) | gidxB (tB*8) | dcB (tB*2) ]
    so agg-2's pass A DMAs cols [0, tA*10) and pass B the rest.
    Slot s of a region maps to gathered-tile position (partition s%128,
    tile s//128).  Padding slots use gather index 0 and dst 999.
    Layer 3: compact table; epack3 keeps the v1 layout
    [gidx3 | dloc3 | dstcol3] with regions split at half the compact
    table."""
    src = np.asarray(edge_src).astype(np.int64)
    dst = np.asarray(edge_dst).astype(np.int64)
    R, NB = cfg.rows, cfg.nb
    RA, RB = cfg.ra, cfg.rb
    N = cfg.n_nodes

    core = dst // R
    blk = (dst - core * R) // P
    s_core = src // R
    s_loc = src - s_core * R
    reg = (s_loc >= RA).astype(np.int64)
    gval = np.where(reg == 0, s_core * RA + s_loc,
                    s_core * RB + (s_loc - RA))

    key = (core * NB + blk) * 2 + reg
    order = np.argsort(key, kind="stable")
    ks, gs, ds = key[order], gval[order], dst[order]
    starts = np.searchsorted(ks, np.arange(2 * NCORES * NB))
    pos = np.arange(len(ks)) - starts[ks]

    nA = np.zeros((NCORES, NB), np.int64)
    nB = np.zeros((NCORES, NB), np.int64)
    np.add.at(nA, (core, blk), 1 - reg)
    np.add.at(nB, (core, blk), reg)
    TA = max(1, _ceil(int(nA.max()), P))
    TB = max(1, _ceil(int(nB.max()), P))

    c_s = ks // (2 * NB)
    b_s = (ks // 2) % NB
    r_s = ks % 2

    gidxA = np.zeros((NCORES, NB, 16, TA * 8), np.int16)
    gidxB = np.zeros((NCORES, NB, 16, TB * 8), np.int16)
    dcA = np.full((NCORES, NB, P, TA), 999, np.int32)
    dcB = np.full((NCORES, NB, P, TB), 999, np.int32)

    mA = r_s == 0
    gidxA[c_s[mA], b_s[mA], pos[mA] % 16, pos[mA] // 16] = \
        gs[mA].astype(np.int16)
    gidxB[c_s[~mA], b_s[~mA], pos[~mA] % 16, pos[~mA] // 16] = \
        gs[~mA].astype(np.int16)
    rloc = ds - c_s * R
    dcA[c_s[mA], b_s[mA], pos[mA] % P, pos[mA] // P] = \
        (rloc[mA] % P).astype(np.int32)
    dcB[c_s[~mA], b_s[~mA], pos[~mA] % P, pos[~mA] // P] = \
        (rloc[~mA] % P).astype(np.int32)

    epack = np.concatenate(
        [np.tile(gidxA, (1, 1, 8, 1)), dcA.view(np.int16).reshape(
            NCORES, NB, P, TA * 2),
         np.tile(gidxB, (1, 1, 8, 1)), dcB.view(np.int16).reshape(
            NCORES, NB, P, TB * 2)], axis=-1)

    # ---- layer-3: compact table of rows referenced by z-edges ----
    nodes = np.arange(N)
    zmask_node = (nodes % cfg.graph) == 0
    zcounts = np.bincount(nodes[zmask_node] // R, minlength=NCORES)
    zslot_of = np.full(N, -1, np.int64)
    for c in range(NCORES):
        zn = nodes[zmask_node & (nodes // R == c)]
        zslot_of[zn] = np.arange(len(zn))

    sel = zmask_node[dst]
    s3, d3 = src[sel], dst[sel]
    # needed local rows per source core (dedup), compact slot assignment
    s3core = s3 // R
    s3loc = s3 - s3core * R
    need_lists = []
    np3 = 0
    for c in range(NCORES):
        u = np.unique(s3loc[s3core == c])
        need_lists.append(u)
        np3 = max(np3, len(u))
    NP3 = _ceil(np3, P) * P
    cmpidx16 = np.zeros((NCORES, 16, NP3 // 16), np.int16)
    cslot_of = np.zeros((NCORES, R), np.int64)
    for c in range(NCORES):
        u = need_lists[c]
        cmpidx16[c, np.arange(len(u)) % 16, np.arange(len(u)) // 16] = \
            u.astype(np.int16)
        cslot_of[c, u] = np.arange(len(u))
    cmpidx = np.tile(cmpidx16, (1, 8, 1))
    # compact global row of each z-edge source
    g3 = s3core * NP3 + cslot_of[s3core, s3loc]
    SPLIT3 = 4 * NP3
    assert NCORES * NP3 - SPLIT3 - 1 <= 32767 and SPLIT3 - 1 <= 32767

    c3 = d3 // R
    r3 = (g3 >= SPLIT3).astype(np.int64)
    key3 = c3 * 2 + r3
    o3 = np.argsort(key3, kind="stable")
    k3, g3o, d3o = key3[o3], g3[o3], d3[o3]
    starts3 = np.searchsorted(k3, np.arange(2 * NCORES))
    pos3 = np.arange(len(k3)) - starts3[k3]
    n3A = np.zeros(NCORES, np.int64)
    n3B = np.zeros(NCORES, np.int64)
    np.add.at(n3A, c3, 1 - r3)
    np.add.at(n3B, c3, r3)
    T3A = max(1, _ceil(int(n3A.max()), P))
    T3B = max(1, _ceil(int(n3B.max()), P))
    T3 = T3A + T3B

    cc3 = k3 // 2
    rr3 = k3 % 2
    slot3 = np.where(rr3 == 0, pos3, T3A * P + pos3)
    gidx316 = np.zeros((NCORES, 16, T3 * 8), np.int16)
    dloc316 = np.zeros((NCORES, 16, T3 * 8), np.int16)
    dstcol3 = np.full((NCORES, P, T3), 999, np.int32)
    wcol3 = np.where(rr3 == 0, pos3 // 16, T3A * 8 + pos3 // 16)
    val3 = np.where(rr3 == 0, g3o, g3o - SPLIT3).astype(np.int16)
    gidx316[cc3, pos3 % 16, wcol3] = val3
    rl3 = d3o - cc3 * R
    dloc316[cc3, slot3 % 16, slot3 // 16] = rl3.astype(np.int16)
    dstcol3[cc3, slot3 % P, slot3 // P] = zslot_of[d3o].astype(np.int32)
    epack3 = np.concatenate(
        [np.tile(gidx316, (1, 8, 1)), np.tile(dloc316, (1, 8, 1)),
         dstcol3.view(np.int16).reshape(NCORES, P, T3 * 2)], axis=-1)

    return dict(TA=TA, TB=TB, T3A=T3A, T3B=T3B, NP3=NP3,
                epack=epack, epack3=epack3, cmpidx=cmpidx,
                zcounts=zcounts)


# ----------------------------------------------------------------------------
# program builder
# ----------------------------------------------------------------------------

def build_program(cfg: Cfg, TA, TB, T3A, T3B, NP3, repeat=1):
    NB, R = cfg.nb, cfg.rows
    NBA = cfg.nba
    RA, RB = cfg.ra, cfg.rb
    NTA, NTB = cfg.na, cfg.nbt
    T, T3 = TA + TB, T3A + T3B
    ROW, ROW3 = cfg.row12, cfg.row3
    HID, OUT = cfg.hid, cfg.out_ch
    SPLIT3 = 4 * NP3
    NBTOT = NB * NCORES

    nc = bacc.Bacc("TRN2", target_bir_lowering=False, debug=False,
                   num_devices=NCORES)

    # ---- I/O ----
    xTf = nc.dram_tensor("xTf", [P, NBTOT * P], F32, kind="ExternalInput")
    xTl = nc.dram_tensor("xTl", [P, R], F32, kind="ExternalInput")
    W1e = nc.dram_tensor("W1e", [P, HID + 3], F32, kind="ExternalInput")
    W2e = nc.dram_tensor("W2e", [HID, HID + 3], F32, kind="ExternalInput")
    W3e = nc.dram_tensor("W3e", [HID, OUT + 3], F32, kind="ExternalInput")
    b1 = nc.dram_tensor("b1", [1, HID], F32, kind="ExternalInput")
    b2 = nc.dram_tensor("b2", [1, HID], F32, kind="ExternalInput")
    b3 = nc.dram_tensor("b3", [1, OUT], F32, kind="ExternalInput")
    epack = nc.dram_tensor("epack", [NB, P, T * 10], I16,
                           kind="ExternalInput")
    epack3 = nc.dram_tensor("epack3", [P, T3 * 18], I16,
                            kind="ExternalInput")
    cmpidx = nc.dram_tensor("cmpidx", [P, NP3 // 16], I16,
                            kind="ExternalInput")
    out_d = nc.dram_tensor("out", [P, OUT], F32, kind="ExternalOutput")

    # ---- internal DRAM ----
    h1tabA = nc.dram_tensor("h1tabA", [NTA, ROW], F32)
    h1tabB = nc.dram_tensor("h1tabB", [NTB, ROW], F32)
    h2shardA = nc.dram_tensor("h2shardA", [RA, ROW], F32)
    h2shardB = nc.dram_tensor("h2shardB", [RB, ROW], F32)
    h2tabA = nc.dram_tensor("h2tabA", [NTA, ROW], F32, addr_space="Shared")
    h2tabB = nc.dram_tensor("h2tabB", [NTB, ROW], F32, addr_space="Shared")
    hs3 = nc.dram_tensor("hs3", [R, 2 * ROW3], F32)
    h3cshard = nc.dram_tensor("h3cshard", [NP3, ROW3], F32)
    h3ctab = nc.dram_tensor("h3ctab", [NCORES * NP3, ROW3], F32,
                            addr_space="Shared")
    stabT1 = nc.dram_tensor("stabT1", [1, NB * P], F32)
    stabT2 = nc.dram_tensor("stabT2", [1, NB * P], F32)
    acc_d = nc.dram_tensor("acc_d", [R, HID + 1], F32)

    rg = [list(range(NCORES))]

    with tile.TileContext(nc) as tc, ExitStack() as ctx:
        cpool = ctx.enter_context(tc.tile_pool(name="const", bufs=1))
        wpool = ctx.enter_context(tc.tile_pool(name="weights", bufs=1))
        xf_pool = ctx.enter_context(tc.tile_pool(name="xchunk", bufs=2))
        lt_pool = ctx.enter_context(tc.tile_pool(name="lhsT", bufs=4))
        d1_pool = ctx.enter_context(tc.tile_pool(name="d1row", bufs=3))
        row_pool = ctx.enter_context(tc.tile_pool(name="rows", bufs=2))
        idx_pool = ctx.enter_context(tc.tile_pool(name="idx", bufs=4))
        g_pool = ctx.enter_context(tc.tile_pool(name="gather", bufs=3))
        s_pool = ctx.enter_context(tc.tile_pool(name="scal", bufs=4))
        se_pool = ctx.enter_context(tc.tile_pool(name="sew", bufs=3))
        tmp_pool = ctx.enter_context(tc.tile_pool(name="tmp", bufs=3))
        a_pool = ctx.enter_context(tc.tile_pool(name="arow", bufs=3))
        bnd_pool = ctx.enter_context(tc.tile_pool(name="band", bufs=2))
        sdc_pool = ctx.enter_context(tc.tile_pool(name="sdc", bufs=2))
        aio_pool = ctx.enter_context(tc.tile_pool(name="aio", bufs=3))
        ps_dense = ctx.enter_context(
            tc.tile_pool(name="psd", bufs=2, space="PSUM"))
        ps_agg = ctx.enter_context(
            tc.tile_pool(name="psa", bufs=3, space="PSUM"))
        ps_tp = ctx.enter_context(
            tc.tile_pool(name="pst", bufs=2, space="PSUM"))
        ps_bc = ctx.enter_context(
            tc.tile_pool(name="psb", bufs=1, space="PSUM"))

        # constants
        ident = cpool.tile([P, P], F32, tag="ident")
        make_identity(nc, ident[:])
        TMAX = max(T, T3)
        iota_i = cpool.tile([P, TMAX * P], I32, tag="iotai")
        nc.gpsimd.iota(iota_i[:], pattern=[[0, TMAX], [1, P]], base=0,
                       channel_multiplier=0)
        ones1 = cpool.tile([1, P], F32, tag="ones1")
        nc.vector.memset(ones1[:], 1.0)

        # preload weights
        w1_sb = wpool.tile([P, HID + 3], F32, tag="w1")
        nc.sync.dma_start(out=w1_sb[:], in_=W1e[:, :])
        w2_sb = [wpool.tile([P, HID + 3], F32, tag=f"w2_{k}",
                            name=f"w2sb{k}") for k in range(2)]
        for k in range(2):
            nc.sync.dma_start(out=w2_sb[k][:], in_=W2e[k * P:(k + 1) * P, :])
        w3_sb = [wpool.tile([P, OUT + 3], F32, tag=f"w3_{k}",
                            name=f"w3sb{k}") for k in range(2)]
        for k in range(2):
            nc.sync.dma_start(out=w3_sb[k][:], in_=W3e[k * P:(k + 1) * P, :])

        def bias_bcast(bd, C, tag):
            brow = cpool.tile([1, C], F32, tag=f"brow_{tag}")
            nc.sync.dma_start(out=brow[:], in_=bd[:, :])
            bps = ps_bc.tile([P, C], F32, tag="bps")
            nc.tensor.matmul(bps[:], lhsT=ones1[:], rhs=brow[:],
                             start=True, stop=True)
            bbc = cpool.tile([P, C], F32, tag=f"bbc_{tag}")
            nc.vector.tensor_copy(out=bbc[:], in_=bps[:])
            return bbc

        def pack_row(row_ap, ps, ncols):
            """row = [h | 1 | s_src | 0-pad]; the constant columns (1.0
            and the zero pad) are pre-initialised in the buffers."""
            nc.vector.tensor_copy(out=row_ap[:, 0:ncols],
                                  in_=ps[:, 0:ncols])
            nc.vector.tensor_copy(out=row_ap[:, ncols + 1:ncols + 2],
                                  in_=ps[:, ncols + 1:ncols + 2])

        def init_row_buf(pool, tag, cols, ncols, nsub, sub):
            """Pre-set constant columns in every rotating buffer."""
            for _ in range(pool.bufs):
                tl = pool.tile([P, cols], F32, tag=tag)
                nc.vector.memset(tl[:], 0.0)
                for i in range(nsub):
                    nc.vector.memset(
                        tl[:, i * sub + ncols:i * sub + ncols + 1], 1.0)

        # s_dst-row staging: collect [P,1] columns for SG consecutive
        # blocks, then transpose and DMA SG rows of stabT at once.
        SG = 8

        def sd_stage(sdc, i, ps_col):
            nc.vector.tensor_copy(out=sdc[:, i:i + 1], in_=ps_col)

        def sd_flush(sdc, stabT, g0, gn):
            tp = ps_tp.tile([P, P], F32, tag="tp")
            nc.tensor.transpose(tp[0:SG, :], sdc[:, 0:SG], ident[:])
            sr = sdc_pool.tile([SG, P], F32, tag="srows")
            nc.scalar.copy(out=sr[0:gn, :], in_=tp[0:gn, :])
            nc.sync.dma_start(
                out=stabT[0:1, g0 * P:(g0 + gn) * P].rearrange(
                    "o (a b) -> (o a) b", b=P),
                in_=sr[0:gn, :])

        # ------------------------------------------------------------------
        def dense1_mini():
            """Sharded pass: stabT1 (core-local per-block s_dst rows)."""
            XC = 10
            xch = None
            for g0 in range(0, NB, SG):
                gn = min(SG, NB - g0)
                sdc = sdc_pool.tile([P, SG], F32, tag="sdc")
                for i in range(gn):
                    it = g0 + i
                    if it % XC == 0:
                        cn = min(XC, NB - it)
                        xch = xf_pool.tile([P, XC * P], F32, tag="xch")
                        nc.sync.dma_start(
                            out=xch[:, 0:cn * P],
                            in_=xTl[:, it * P:(it + cn) * P])
                    xo = (it % XC) * P
                    ps = ps_dense.tile([P, 1], F32, tag="dps")
                    nc.tensor.matmul(ps[:], lhsT=xch[:, xo:xo + P],
                                     rhs=w1_sb[:, HID + 2:HID + 3],
                                     start=True, stop=True)
                    sd_stage(sdc, i, ps[:, 0:1])
                sd_flush(sdc, stabT1, g0, gn)

        # ------------------------------------------------------------------
        def dense1_full():
            """Replicated dense-1: every core computes the whole table."""
            XC = 10  # blocks per xTf chunk load
            DG = 5   # blocks per table-row write group
            for c0 in range(0, NBTOT, XC):
                cn = min(XC, NBTOT - c0)
                xch = xf_pool.tile([P, XC * P], F32, tag="xch")
                nc.sync.dma_start(out=xch[:, 0:cn * P],
                                  in_=xTf[:, c0 * P:(c0 + cn) * P])
                g = c0
                while g < c0 + cn:
                    core_i = g // NB
                    bl = g % NB
                    if bl < NBA:
                        gn = min(NBA - bl, c0 + cn - g, DG)
                    else:
                        gn = min(NB - bl, c0 + cn - g, DG)
                    rowt = d1_pool.tile([P, DG * ROW], F32, tag="d1row")
                    for i in range(gn):
                        it = g + i
                        ps = ps_dense.tile([P, HID + 3], F32, tag="dps")
                        nc.tensor.matmul(
                            ps[:],
                            lhsT=xch[:, (it - c0) * P:(it - c0 + 1) * P],
                            rhs=w1_sb[:], start=True, stop=True)
                        pack_row(rowt[:, i * ROW:(i + 1) * ROW], ps, HID)
                    if bl < NBA:
                        dst = h1tabA[(core_i * NBA + bl) * P:
                                     (core_i * NBA + bl + gn) * P, :]
                    else:
                        b_off = core_i * (NB - NBA) + (bl - NBA)
                        dst = h1tabB[b_off * P:(b_off + gn) * P, :]
                    nc.sync.dma_start(
                        out=dst.rearrange("(c p) f -> p c f", p=P),
                        in_=rowt[:, 0:gn * ROW].rearrange(
                            "p (c f) -> p c f", f=ROW))
                    g += gn

        # ------------------------------------------------------------------
        BG = 4  # blocks per band-broadcast matmul

        def make_bandg(stabT, g0):
            gw = min(BG, NB - g0)
            srow_sb = bnd_pool.tile([1, BG * P], F32, tag="srow")
            nc.sync.dma_start(out=srow_sb[:, 0:gw * P],
                              in_=stabT[0:1, g0 * P:(g0 + gw) * P])
            bps = ps_bc.tile([P, BG * P], F32, tag="bps")
            nc.tensor.matmul(bps[:, 0:gw * P], lhsT=ones1[:],
                             rhs=srow_sb[:, 0:gw * P], start=True, stop=True)
            bandg = bnd_pool.tile([P, BG * P], F32, tag="bandg")
            nc.vector.tensor_copy(out=bandg[:, 0:gw * P],
                                  in_=bps[:, 0:gw * P])
            return bandg

        def region_pass(G, swa, nt, t0, gi, dcs, band, tab, rowlen, n_mm,
                        ps, start, stop):
            """Gather region tiles, build Se_w, run the matmuls.

            G/swa are tile slices sized for nt tiles; t0 is the iota
            tile offset (region B starts at tA for the combined agg-1
            pass so slot ids match dc values built per-region)."""
            G3d = G.rearrange("p (t c) -> p t c", c=rowlen)
            nc.gpsimd.dma_gather(
                out_ap=G3d, in_ap=tab, idxs_ap=gi, num_idxs=nt * P,
                num_idxs_reg=nt * P, elem_size=rowlen, elem_step=rowlen,
                single_packet=False)
            swa3 = swa.rearrange("p (t d) -> p t d", d=P)
            dc3 = dcs.unsqueeze(-1).to_broadcast([P, nt, P])
            nc.vector.tensor_tensor(
                out=swa3, in0=iota_i[:, 0:nt * P].rearrange(
                    "p (t d) -> p t d", d=P),
                in1=dc3, op=mybir.AluOpType.is_equal)
            tmp = tmp_pool.tile([P, T * P], F32, tag="tmp")
            tmp3 = tmp[:, 0:nt * P].rearrange("p (t d) -> p t d", d=P)
            band3 = band[:].unsqueeze(1).to_broadcast([P, nt, P])
            nc.vector.tensor_tensor(out=tmp3, in0=swa3, in1=band3,
                                    op=mybir.AluOpType.mult)
            sd = s_pool.tile([P, T], F32, tag="sd")
            nc.vector.tensor_reduce(out=sd[:, 0:nt], in_=tmp3,
                                    axis=mybir.AxisListType.X,
                                    op=mybir.AluOpType.add)
            ssrc = G[:, n_mm::rowlen]
            z = s_pool.tile([P, T], F32, tag="z")
            nc.vector.tensor_tensor(out=z[:, 0:nt], in0=ssrc,
                                    in1=sd[:, 0:nt],
                                    op=mybir.AluOpType.add)
            e = s_pool.tile([P, T], F32, tag="e")
            nc.vector.scalar_tensor_tensor(
                out=e[:, 0:nt], in0=z[:, 0:nt], scalar=NEG_ATT,
                in1=z[:, 0:nt],
                op0=mybir.AluOpType.mult, op1=mybir.AluOpType.max)
            w = s_pool.tile([P, T], F32, tag="w")
            nc.scalar.activation(w[:, 0:nt], e[:, 0:nt],
                                 mybir.ActivationFunctionType.Exp)
            w3 = w[:, 0:nt].unsqueeze(-1).to_broadcast([P, nt, P])
            nc.vector.tensor_tensor(out=swa3, in0=swa3, in1=w3,
                                    op=mybir.AluOpType.mult)
            for t in range(nt):
                nc.tensor.matmul(
                    ps[:], lhsT=swa[:, t * P:(t + 1) * P],
                    rhs=G[:, t * rowlen:t * rowlen + n_mm],
                    start=(start and t == 0), stop=(stop and t == nt - 1))

        # ------------------------------------------------------------------
        def epilogue(layer, src_ap, bbc, C_out, w_next, next_cols, b,
                     shards, stabT_next, sdc, stab3_t):
            """Softmax divide + bias (+ fused next dense + row pack)."""
            dn = s_pool.tile([P, 1], F32, tag="dn")
            nc.vector.tensor_scalar_add(dn[:], src_ap[:, C_out:C_out + 1],
                                        1e-30)
            rc = s_pool.tile([P, 1], F32, tag="rc")
            nc.vector.reciprocal(rc[:], dn[:])
            ar = a_pool.tile([P, C_out], F32, tag="ar")
            nc.scalar.activation(ar[:], src_ap[:, 0:C_out],
                                 mybir.ActivationFunctionType.Copy,
                                 scale=rc[:])
            nc.vector.tensor_tensor(out=ar[:], in0=ar[:], in1=bbc[:],
                                    op=mybir.AluOpType.add)
            if layer == 3:
                nc.sync.dma_start(out=out_d[:, :], in_=ar[:])
                return
            ar2 = a_pool.tile([P, C_out], F32, tag="ar2")
            nc.vector.scalar_tensor_tensor(
                out=ar2[:], in0=ar[:], scalar=NEG_ACT, in1=ar[:],
                op0=mybir.AluOpType.mult, op1=mybir.AluOpType.max)
            tps = []
            for k in range(2):
                tp = ps_tp.tile([P, P], F32, tag="tp")
                nc.tensor.transpose(tp[:], ar2[:, k * P:(k + 1) * P],
                                    ident[:])
                tps.append(tp)
            lts = []
            for k in range(2):
                lt = lt_pool.tile([P, P], F32, tag="flt")
                nc.scalar.copy(out=lt[:], in_=tps[k][:])
                lts.append(lt)
            psd = ps_dense.tile([P, next_cols + 3], F32, tag="dps")
            for k in range(2):
                nc.tensor.matmul(psd[:], lhsT=lts[k][:], rhs=w_next[k][:],
                                 start=(k == 0), stop=(k == 1))
            if layer == 1:
                row = row_pool.tile([P, ROW], F32, tag="frow")
            else:
                row = row_pool.tile([P, 2 * ROW3], F32, tag="frow2")
            pack_row(row[:], psd, next_cols)
            if layer == 1:
                if b < NBA:
                    nc.sync.dma_start(
                        out=shards[0][b * P:(b + 1) * P, :], in_=row[:])
                else:
                    nc.sync.dma_start(
                        out=shards[1][(b - NBA) * P:(b - NBA + 1) * P, :],
                        in_=row[:])
                sd_stage(sdc, b % SG, psd[:, next_cols + 2:next_cols + 3])
                if b % SG == SG - 1 or b == NB - 1:
                    sd_flush(sdc, stabT_next, (b // SG) * SG,
                             b % SG + 1)
            else:
                nc.vector.tensor_copy(
                    out=row[:, ROW3:ROW3 + 1],
                    in_=psd[:, next_cols + 2:next_cols + 3])
                nc.sync.dma_start(
                    out=shards[0][b * P:(b + 1) * P, :], in_=row[:])

        # pre-init constant columns of rotating row buffers and
        # zero the gather buffers (so -1-padded gather slots read finite
        # stale data instead of uninitialised SBUF)
        init_row_buf(d1_pool, "d1row", 5 * ROW, HID, 5, ROW)
        init_row_buf(row_pool, "frow", ROW, HID, 1, ROW)
        init_row_buf(row_pool, "frow2", 2 * ROW3, OUT, 1,
                     2 * ROW3)
        for _ in range(g_pool.bufs):
            tl = g_pool.tile([P, T * ROW], F32, tag="G")
            nc.vector.memset(tl[:], 0.0)

        # ====================== the network ======================
        for _rep in range(repeat):
            bbc1 = bias_bcast(b1, HID, "b1")
            bbc2 = bias_bcast(b2, HID, "b2")
            bbc3 = bias_bcast(b3, OUT, "b3")
            dense1_mini()
            dense1_full()

            # ---- fused agg-1 + dense-2 ----
            sdc2 = None
            bandg = None
            for b in range(NB):
                if b % SG == 0:
                    sdc2 = sdc_pool.tile([P, SG], F32, tag="sdc")
                if b % BG == 0:
                    bandg = make_bandg(stabT1, b)
                band = bandg[:, (b % BG) * P:(b % BG + 1) * P]
                ep = idx_pool.tile([P, T * 10], I16, tag="ep")
                nc.sync.dma_start(out=ep[:], in_=epack[b, :, :])
                G = g_pool.tile([P, T * ROW], F32, tag="G")
                swa = se_pool.tile([P, T * P], F32, tag="swa")
                ps = ps_agg.tile([P, HID + 1], F32, tag="aps")
                region_pass(G[:, 0:TA * ROW], swa[:, 0:TA * P], TA, 0,
                            ep[:, 0:TA * 8],
                            ep[:, TA * 8:TA * 10].bitcast(I32),
                            band, h1tabA.ap(), ROW, HID + 1, ps,
                            True, False)
                region_pass(G[:, TA * ROW:T * ROW], swa[:, TA * P:T * P],
                            TB, TA, ep[:, TA * 10:TA * 10 + TB * 8],
                            ep[:, TA * 10 + TB * 8:T * 10].bitcast(I32),
                            band, h1tabB.ap(), ROW, HID + 1, ps,
                            False, True)
                epilogue(1, ps[:], bbc1, HID, w2_sb, HID, b,
                         (h2shardA, h2shardB), stabT2, sdc2, None)
                if b == NBA - 1:
                    nc.gpsimd.collective_compute(
                        "AllGather", mybir.AluOpType.bypass,
                        replica_groups=rg,
                        ins=[h2shardA.ap()], outs=[h2tabA.ap()])
            nc.gpsimd.collective_compute(
                "AllGather", mybir.AluOpType.bypass, replica_groups=rg,
                ins=[h2shardB.ap()], outs=[h2tabB.ap()])

            # ---- agg-2 pass A (table A only; overlaps AG2b) ----
            bandg = None
            for b in range(NB):
                if b % BG == 0:
                    bandg = make_bandg(stabT2, b)
                band = bandg[:, (b % BG) * P:(b % BG + 1) * P]
                ep = idx_pool.tile([P, TA * 10], I16, tag="ep")
                nc.sync.dma_start(out=ep[:], in_=epack[b, :, 0:TA * 10])
                G = g_pool.tile([P, T * ROW], F32, tag="G")
                swa = se_pool.tile([P, T * P], F32, tag="swa")
                ps = ps_agg.tile([P, HID + 1], F32, tag="aps")
                region_pass(G[:, 0:TA * ROW], swa[:, 0:TA * P], TA, 0,
                            ep[:, 0:TA * 8],
                            ep[:, TA * 8:TA * 10].bitcast(I32),
                            band, h2tabA.ap(), ROW, HID + 1, ps,
                            True, True)
                acw = aio_pool.tile([P, HID + 1], F32, tag="acw")
                nc.vector.tensor_copy(out=acw[:], in_=ps[:])
                nc.sync.dma_start(out=acc_d[b * P:(b + 1) * P, :],
                                  in_=acw[:])

            # ---- agg-2 pass B + fused dense-3 ----
            bandg = None
            for b in range(NB):
                if b % BG == 0:
                    bandg = make_bandg(stabT2, b)
                band = bandg[:, (b % BG) * P:(b % BG + 1) * P]
                accb = aio_pool.tile([P, HID + 1], F32, tag="acb")
                nc.sync.dma_start(out=accb[:],
                                  in_=acc_d[b * P:(b + 1) * P, :])
                ep = idx_pool.tile([P, TB * 10], I16, tag="ep")
                nc.sync.dma_start(out=ep[:],
                                  in_=epack[b, :, TA * 10:T * 10])
                G = g_pool.tile([P, T * ROW], F32, tag="G")
                swa = se_pool.tile([P, T * P], F32, tag="swa")
                ps = ps_agg.tile([P, HID + 1], F32, tag="aps")
                region_pass(G[:, 0:TB * ROW], swa[:, 0:TB * P], TB, 0,
                            ep[:, 0:TB * 8],
                            ep[:, TB * 8:TB * 10].bitcast(I32),
                            band, h2tabB.ap(), ROW, HID + 1, ps,
                            True, True)
                arp = a_pool.tile([P, HID + 1], F32, tag="arp")
                nc.vector.tensor_tensor(
                    out=arp[:], in0=ps[:], in1=accb[:],
                    op=mybir.AluOpType.add)
                epilogue(2, arp[:], bbc2, HID, w3_sb, OUT, b,
                         (hs3,), None, None, None)

            # ---- compact + AllGather layer-3 table ----
            ci = idx_pool.tile([P, NP3 // 16], I16, tag="ci")
            nc.sync.dma_start(out=ci[:], in_=cmpidx[:, :])
            cmp_sb = g_pool.tile([P, (NP3 // P) * ROW3], F32, tag="G")
            cmp3d = cmp_sb[:].rearrange("p (t c) -> p t c", c=ROW3)
            nc.gpsimd.dma_gather(
                out_ap=cmp3d, in_ap=hs3[:, 0:ROW3], idxs_ap=ci[:],
                num_idxs=NP3, num_idxs_reg=NP3, elem_size=ROW3,
                elem_step=2 * ROW3, single_packet=False)
            nc.sync.dma_start(
                out=h3cshard.ap().rearrange("(t p) f -> p t f", p=P),
                in_=cmp3d)
            # agg-3 inputs that do not depend on the collective
            ep = idx_pool.tile([P, T3 * 18], I16, tag="ep3")
            nc.sync.dma_start(out=ep[:], in_=epack3[:, :])
            gi = ep[:, 0:T3 * 8]
            dl = ep[:, T3 * 8:T3 * 16]
            dc = ep[:, T3 * 16:T3 * 18].bitcast(I32)
            Gs = tmp_pool.tile([P, T3 * SROW], F32, tag="tmp")
            Gs3d = Gs[:, 0:T3 * SROW].rearrange("p (t c) -> p t c", c=SROW)
            nc.gpsimd.dma_gather(
                out_ap=Gs3d, in_ap=hs3[:, ROW3:2 * ROW3], idxs_ap=dl,
                num_idxs=T3 * P, num_idxs_reg=T3 * P, elem_size=SROW,
                elem_step=2 * ROW3, single_packet=False)
            nc.gpsimd.collective_compute(
                "AllGather", mybir.AluOpType.bypass, replica_groups=rg,
                ins=[h3cshard.ap()], outs=[h3ctab.ap()])

            # ---- agg-3 (single block) ----
            G = g_pool.tile([P, T3 * ROW3], F32, tag="G")
            G3d = G[:].rearrange("p (t c) -> p t c", c=ROW3)
            nc.gpsimd.dma_gather(
                out_ap=G3d[:, 0:T3A, :], in_ap=h3ctab[0:SPLIT3, :],
                idxs_ap=gi[:, 0:T3A * 8], num_idxs=T3A * P,
                num_idxs_reg=T3A * P, elem_size=ROW3, elem_step=ROW3,
                single_packet=False)
            nc.gpsimd.dma_gather(
                out_ap=G3d[:, T3A:T3, :],
                in_ap=h3ctab[SPLIT3:NCORES * NP3, :],
                idxs_ap=gi[:, T3A * 8:T3 * 8], num_idxs=T3B * P,
                num_idxs_reg=T3B * P, elem_size=ROW3, elem_step=ROW3,
                single_packet=False)
            sdp = Gs[:, 0:T3 * SROW:SROW]
            ssrc = G[:, OUT + 1::ROW3]
            z = s_pool.tile([P, T3], F32, tag="z")
            nc.vector.tensor_tensor(out=z[:], in0=ssrc, in1=sdp,
                                    op=mybir.AluOpType.add)
            e = s_pool.tile([P, T3], F32, tag="e")
            nc.vector.scalar_tensor_tensor(
                out=e[:], in0=z[:], scalar=NEG_ATT, in1=z[:],
                op0=mybir.AluOpType.mult, op1=mybir.AluOpType.max)
            w = s_pool.tile([P, T3], F32, tag="w")
            nc.scalar.activation(w[:], e[:],
                                 mybir.ActivationFunctionType.Exp)
            ps = ps_agg.tile([P, OUT + 1], F32, tag="aps")
            swa = se_pool.tile([P, T3 * P], F32, tag="swa")
            dc3 = dc.unsqueeze(-1).to_broadcast([P, T3, P])
            w3b = w[:].unsqueeze(-1).to_broadcast([P, T3, P])
            swa3 = swa[:].rearrange("p (t d) -> p t d", d=P)
            nc.vector.tensor_tensor(
                out=swa3, in0=iota_i[:, 0:T3 * P].rearrange(
                    "p (t d) -> p t d", d=P),
                in1=dc3, op=mybir.AluOpType.is_equal)
            nc.vector.tensor_tensor(out=swa3, in0=swa3, in1=w3b,
                                    op=mybir.AluOpType.mult)
            for t in range(T3):
                nc.tensor.matmul(
                    ps[:], lhsT=swa[:, t * P:(t + 1) * P],
                    rhs=G[:, t * ROW3:t * ROW3 + OUT + 1],
                    start=(t == 0), stop=(t == T3 - 1))
            epilogue(3, ps[:], bbc3, OUT, None, 0, 0, None, None, None,
                     None)

    nc.compile()
    return nc


# ----------------------------------------------------------------------------
# host wrapper
# ----------------------------------------------------------------------------

def make_in_maps(inputs, pre, cfg: Cfg):
    R = cfg.rows
    N = cfg.n_nodes
    NTOT = cfg.ntot
    x = np.asarray(inputs["x"], np.float32)

    def wext(W, a_s, a_d):
        W = np.asarray(W, np.float32)
        a_s = np.asarray(a_s, np.float32)
        a_d = np.asarray(a_d, np.float32)
        z = np.zeros((W.shape[0], 1), np.float32)
        return np.concatenate(
            [W, z, (W @ a_s)[:, None], (W @ a_d)[:, None]], axis=1
        ).astype(np.float32)

    W1e = wext(inputs["W1"], inputs["a_src1"], inputs["a_dst1"])
    W2e = wext(inputs["W2"], inputs["a_src2"], inputs["a_dst2"])
    W3e = wext(inputs["W3"], inputs["a_src3"], inputs["a_dst3"])
    b1 = np.asarray(inputs["b1"], np.float32).reshape(1, -1)
    b2 = np.asarray(inputs["b2"], np.float32).reshape(1, -1)
    b3 = np.asarray(inputs["b3"], np.float32).reshape(1, -1)
    xf = np.zeros((P, NTOT), np.float32)
    xf[:, 0:N] = x.T
    in_maps = []
    for c in range(NCORES):
        lo, hi = c * R, min((c + 1) * R, N)
        xs = np.zeros((P, R), np.float32)
        xs[:, 0:max(0, hi - lo)] = x[lo:hi].T
        in_maps.append({
            "xTf": xf, "xTl": xs, "W1e": W1e, "W2e": W2e, "W3e": W3e,
            "b1": b1, "b2": b2, "b3": b3,
            "epack": pre["epack"][c], "epack3": pre["epack3"][c],
            "cmpidx": pre["cmpidx"][c],
        })
    return in_maps


_CACHE = {}


def get_program(cfg: Cfg, TA, TB, T3A, T3B, NP3, repeat=1):
    key = (cfg.n_nodes, TA, TB, T3A, T3B, NP3, repeat)
    if key not in _CACHE:
        _CACHE[key] = build_program(cfg, TA, TB, T3A, T3B, NP3, repeat)
    return _CACHE[key]


def run(inputs, cfg: Cfg, trace=False):
    pre = preprocess(inputs["edge_src"], inputs["edge_dst"], cfg)
    in_maps = make_in_maps(inputs, pre, cfg)
    nc = get_program(cfg, pre["TA"], pre["TB"], pre["T3A"], pre["T3B"],
                     pre["NP3"])
    res = run_bass_kernel_spmd(nc, in_maps, list(range(NCORES)), trace=trace)
    outs = []
    for c in range(NCORES):
        outs.append(res.results[c]["out"][0:pre["zcounts"][c], :])
    return np.concatenate(outs, axis=0).astype(np.float32), res


def kernel(**inputs):
    cfg = Cfg(n_nodes=inputs["x"].shape[0],
              in_ch=inputs["x"].shape[1],
              hid=inputs["W1"].shape[1],
              out_ch=inputs["W3"].shape[1])
    out, _ = run(inputs, cfg)
    return out
